# revision 22
# baseline (speedup 1.0000x reference)
"""Trainium2 Bass kernel for the BasicQuadRGBV2 demosaic model.

Data-parallel over batch: 1 image per NeuronCore (8 cores).

Per-core dataflow (image [4,512,512] -> [3,1024,1024]):
  Phase 1  (conv stacks): two 3-layer CNNs (4->12->12->12, 3x3, relu) computed
           as block-banded bf16 matmuls. Layout: partitions = (y_row_window x
           chan), free dim = x. The y-taps of each 3x3 conv live inside a
           banded lhsT (contract over (y_in, c)); the x-taps are 3
           PSUM-accumulated matmuls over free-dim-shifted views. Strips of 8
           output rows; the output grid drifts +1 row per layer so PSUM
           evictions always land at natural partitions; strip-to-strip halo
           rows move via small DMAs emitted one iteration ahead. The f and w
           stacks share one strip tile (f in cols 0:514, w in 514:1028) so
           each halo is a single DMA.
  Phase 2  (softmax green): E=exp(relu-free w3), i=relu(f3); selector matmuls
           reduce over channels-in-partitions giving g0num/g1num/den planes
           (f32).
  Phase 2.5 (per 64-row half-chunk): rden~=1/den (fast approx); g0,g1 (bf16);
           chroma c1=mosaic1-g0, c2=mosaic2-g1 (bf16).
  Phase 3  (chroma 5x5 convs): in pixel-shuffled space each needed
           (conv, phase) output is a 12-tap stencil over (c1,c2) within a
           3x3 quad-space window -> same banded-matmul machinery, 6 outputs
           at once.
  Phase 4  (assembly): DVE/scalar writes with stride-2 free APs interleave
           quad planes into full-res f32 rows; contiguous row DMAs to DRAM.

Phases 2.5/3/4 are drip-fed into the wavefront via ready-queues (bounded
work per iteration per engine) so the PE never idles long enough for the
HAM clock-gate to drop it to half rate.
"""

import numpy as np
import ml_dtypes

import concourse.bass as bass
import concourse.tile as tile
from concourse import bacc, mybir
from concourse.tile import add_dep_helper as _adh


def add_dep(frm, to, reason=""):
    _adh(frm.ins, to.ins, reason=reason)


from concourse.bass_utils import run_bass_kernel_spmd

F32 = mybir.dt.float32
BF16 = mybir.dt.bfloat16
RELU = mybir.ActivationFunctionType.Relu
EXP = mybir.ActivationFunctionType.Exp
COPY = mybir.ActivationFunctionType.Copy

WIDTH = 12
HW = 512  # image H = W (quad space)
NSTRIP = 65  # strips s = -1 .. 63, stride 8

# wblob column offsets
W1_OFS = 0            # [40, 6*96]   (st,dx)
W23_OFS = 576         # [120, 12*96] (ly,dx)
SEL_OFS = W23_OFS + 1152   # [96, 48]
W5_OFS = SEL_OFS + 48      # [64, 3*96]
WBLOB_COLS = W5_OFS + 288


# ---------------------------------------------------------------- host prep

def _band_lhsT(W, cin):
    """W: [12, cin, 3, 3] -> [3, 10*cin, 96] banded matrices (one per x-tap).

    lhsT_dx[(yi*cin + c), (yo*12 + oc)] = W[oc, c, yi - yo, dx]
    """
    K, M = 10 * cin, 8 * WIDTH
    out = np.zeros((3, K, M), np.float32)
    for dx in range(3):
        for yo in range(8):
            for dy in range(3):
                yi = yo + dy
                out[dx, yi * cin:(yi + 1) * cin, yo * WIDTH:(yo + 1) * WIDTH] = \
                    W[:, :, dy, dx].T
    return out


def _selectors():
    selA = np.zeros((96, 24), np.float32)  # applied to i*E
    selB = np.zeros((96, 24), np.float32)  # applied to E
    for yl in range(8):
        for c in range(WIDTH):
            p = yl * WIDTH + c
            if c < 6:
                selA[p, yl * 3 + 0] = 1.0
            else:
                selA[p, yl * 3 + 1] = 1.0
            selB[p, yl * 3 + 2] = 1.0
    return selA, selB


def _g_stencil(K5, py, px):
    """12-tap quad-space stencil of a 5x5 conv output at phase (py,px),
    over chroma channels c1 (phase (0,1)) and c2 (phase (1,0))."""
    G = np.zeros((2, 3, 3), np.float32)
    for cc, (qy, qx) in enumerate(((0, 1), (1, 0))):
        for dy in (-1, 0, 1):
            for dx in (-1, 0, 1):
                d5y = 2 * dy + 2 - py + qy
                d5x = 2 * dx + 2 - px + qx
                if 0 <= d5y < 5 and 0 <= d5x < 5:
                    G[cc, dy + 1, dx + 1] = K5[d5y, d5x]
    return G


def _chroma_lhsT(chw, cvw, cqw):
    """-> [3, 64, 96] banded matrices for the 6 (conv, phase) outputs.

    Output order o: 0 ch@(0,0), 1 ch@(1,1), 2 cv@(0,0), 3 cv@(1,1),
                    4 cq@(1,0), 5 cq@(0,1).
    """
    specs = [(chw, 0, 0), (chw, 1, 1), (cvw, 0, 0), (cvw, 1, 1),
             (cqw, 1, 0), (cqw, 0, 1)]
    out = np.zeros((3, 64, 96), np.float32)
    for o, (K5, py, px) in enumerate(specs):
        G = _g_stencil(np.asarray(K5)[0, 0], py, px)
        for dx in range(3):
            for yo in range(16):
                for dy in (-1, 0, 1):
                    yi = yo + dy + 1
                    for cc in range(2):
                        out[dx, cc * 32 + yi, yo * 6 + o] = G[cc, dy + 1, dx]
    return out


def _host_prep(inputs):
    mosaic = np.asarray(inputs["mosaic"], np.float32)  # [8,4,512,512]
    mospad = np.zeros((mosaic.shape[0], 4, 522, 514), ml_dtypes.bfloat16)
    mospad[:, :, 8:520, 1:513] = mosaic
    wblob = np.zeros((120, WBLOB_COLS), np.float32)
    w1 = [_band_lhsT(np.asarray(inputs["fw0"]), 4),
          _band_lhsT(np.asarray(inputs["ww0"]), 4)]
    for st in range(2):
        for dx in range(3):
            wblob[0:40, W1_OFS + (st * 3 + dx) * 96:
                  W1_OFS + (st * 3 + dx + 1) * 96] = w1[st][dx]
    w23 = [_band_lhsT(np.asarray(inputs["fw1"]), 12),
           _band_lhsT(np.asarray(inputs["ww1"]), 12),
           _band_lhsT(np.asarray(inputs["fw2"]), 12),
           _band_lhsT(np.asarray(inputs["ww2"]), 12)]
    for ly in range(4):
        for dx in range(3):
            wblob[0:120, W23_OFS + (ly * 3 + dx) * 96:
                  W23_OFS + (ly * 3 + dx + 1) * 96] = w23[ly][dx]
    selA, selB = _selectors()
    wblob[0:96, SEL_OFS:SEL_OFS + 24] = selA
    wblob[0:96, SEL_OFS + 24:SEL_OFS + 48] = selB
    w5 = _chroma_lhsT(inputs["chw"], inputs["cvw"], inputs["cqw"])
    for dx in range(3):
        wblob[0:64, W5_OFS + dx * 96:W5_OFS + (dx + 1) * 96] = w5[dx]
    return mospad, {"wblob": wblob.astype(ml_dtypes.bfloat16)}


# ---------------------------------------------------------------- kernel IR

def build_kernel(tc, outs, ins, ctx):
    nc = tc.nc
    mospad, wblob = ins["mospad"], ins["wblob"]
    out = outs["out"]

    wp = ctx.enter_context(tc.tile_pool(name="weights", bufs=1))
    pp = ctx.enter_context(tc.tile_pool(name="planes", bufs=1))
    # conv layer pairs accumulate f|w into one 2-bank PSUM tile -> single
    # paired eviction ACTIVATE; 3 pairs (L1,L2,L3) + p2 + p3 = 8 banks
    psp = ctx.enter_context(tc.tile_pool(name="psp", bufs=3, space="PSUM"))
    ps2 = ctx.enter_context(tc.tile_pool(name="ps2", bufs=1, space="PSUM"))
    ps3 = ctx.enter_context(tc.tile_pool(name="ps3", bufs=1, space="PSUM"))
    pools = {}
    for tag, b in (("b0", 4), ("b1", 4), ("b2", 4), ("b3", 4)):
        pools[tag] = ctx.enter_context(tc.tile_pool(name=f"p_{tag}", bufs=b))
    ph2 = ctx.enter_context(tc.tile_pool(name="ph2", bufs=3))
    qpp = ctx.enter_context(tc.tile_pool(name="qp", bufs=2))
    asmp = ctx.enter_context(tc.tile_pool(name="asm", bufs=4))

    # --- weights to SBUF (single DMA)
    wb = wp.tile([120, WBLOB_COLS], BF16, tag="wb")
    nc.sync.dma_start(wb[:], wblob)

    def w1_v(st, dx):
        return wb[0:40, W1_OFS + (st * 3 + dx) * 96:W1_OFS + (st * 3 + dx + 1) * 96]

    def w23_v(ly, dx):
        return wb[0:120, W23_OFS + (ly * 3 + dx) * 96:
                  W23_OFS + (ly * 3 + dx + 1) * 96]

    selA_v = wb[0:96, SEL_OFS:SEL_OFS + 24]
    selB_v = wb[0:96, SEL_OFS + 24:SEL_OFS + 48]

    def w5_v(dx):
        return wb[0:64, W5_OFS + dx * 96:W5_OFS + (dx + 1) * 96]

    # --- persistent planes [128, 2048]: y -> (y%128, (y//128)*512 + x)
    mosp = pp.tile([128, 4 * 2048], BF16, tag="mosp")
    for c in range(4):
        nc.sync.dma_start(
            mosp[:, c * 2048:(c + 1) * 2048].rearrange(
                "p (t x) -> p t x", t=4),
            mospad[c, 8:520, 1:513].rearrange("(t p) x -> p t x", p=128))

    def mos_pl(c):
        return mosp[:, c * 2048:(c + 1) * 2048]

    g3 = pp.tile([128, 3 * 2048], F32, tag="g3")
    g0n = g3[:, 0:2048]
    g1n = g3[:, 2048:4096]
    den = g3[:, 4096:6144]
    g01 = pp.tile([128, 2 * 2048], BF16, tag="g01")
    g0p = g01[:, 0:2048]
    g1p = g01[:, 2048:4096]
    c12 = pp.tile([128, 2 * 2048], BF16, tag="c12")
    c1p = c12[:, 0:2048]
    c2p = c12[:, 2048:4096]
    rden = pp.tile([128, 512], F32, tag="rden")
    zt = pp.tile([96, 1028], BF16, tag="zt")
    nc.gpsimd.memset(zt[:], 0.0)

    # --- phase 1+2 wavefront over strips
    b0_t, b1_t, b2_t = {}, {}, {}

    def load_b0(s):
        t = pools["b0"].tile([40, 514], BF16, tag="b0", name="b0")
        src = mospad[:, 8 * s + 8:8 * s + 18, :].transpose([1, 0, 2])
        d = nc.sync.dma_start(t[:], src)
        b0_t[s] = (t, [d])

    def conv_pair(rhs_tile, kdim, wvf, wvw, cofs_w, deps=()):
        # f stack -> psum cols 0:512, w stack -> 512:1024 (adjacent banks)
        pt = psp.tile([96, 1024], F32, tag="cp2", name="cp2")
        mms = []
        for half, (wv, cofs) in enumerate(((wvf, 0), (wvw, cofs_w))):
            for dx in range(3):
                mms.append(nc.tensor.matmul(
                    pt[:, half * 512:half * 512 + 512], wv(dx),
                    rhs_tile[0:kdim, cofs + dx:cofs + dx + 512],
                    start=(dx == 0), stop=(dx == 2)))
        for mm in mms:
            for dep in deps:
                add_dep(mm, dep, reason="rhs-ready")
        return pt

    def evict_pair(pt, store, s, tag, k):
        # strip rows m=0..7 hold y = 8s+k+m; rows outside [0,512) must be
        # exactly zero (conv zero-padding) or they leak into the next layer.
        # f stack -> cols 1:513, w stack -> 515:1027 of one tile, written by
        # a single two-segment ACTIVATE from the paired PSUM.
        t = pools[tag].tile([120, 1028], BF16, tag=tag, name=tag)
        a1 = nc.scalar.activation(
            t[0:96, :].rearrange("p (h x) -> p h x", h=2)[:, :, 1:513],
            pt[:], RELU)
        z0 = nc.gpsimd.memset(
            t[0:96, :].rearrange("p (h x) -> p h x", h=2)[:, :, 0:514:513], 0.0)
        add_dep(z0, a1, reason="pad-cols")
        insts = [a1, z0]
        if s == -1 and 8 - k > 0:
            z = nc.sync.dma_start(t[0:(8 - k) * 12, :], zt[0:(8 - k) * 12, :])
            for a in (a1, z0):
                add_dep(z, a, reason="zero-pad-rows")
            insts.append(z)
        if s == 63 and 8 - k < 8:
            z = nc.sync.dma_start(t[(8 - k) * 12:96, :], zt[0:k * 12, :])
            for a in (a1, z0):
                add_dep(z, a, reason="zero-pad-rows")
            insts.append(z)
        store[s] = (t, insts)

    def halo(store, s, eng):
        # store[s][96:120] <- store[s+1][0:24]  (rows y+8, y+9)
        dst, insts = store[s]
        if s + 1 in store:
            d = eng.dma_start(dst[96:120, :], store[s + 1][0][0:24, :])
        else:
            d = eng.dma_start(dst[96:120, :], zt[0:24, :])
        for i_ in insts:
            add_dep(d, i_, reason="halo-after-evict")

    def phase2(s, pt):
        it = ph2.tile([96, 512], BF16, tag="i")
        et = ph2.tile([96, 512], BF16, tag="e")
        nc.vector.tensor_scalar_max(it[:], pt[:, 0:512], 0.0)
        nc.scalar.activation(et[:], pt[:, 512:1024], EXP)
        # exp(relu(x)) == max(exp(x), 1) -- this IS the last-layer relu
        nc.gpsimd.tensor_scalar_max(et[:], et[:], 1.0)
        nc.vector.tensor_mul(it[:], it[:], et[:])  # i*E in place
        p2 = ps2.tile([24, 512], F32, tag="p2")
        nc.tensor.matmul(p2[:], selA_v, it[:], start=True, stop=False)
        nc.tensor.matmul(p2[:], selB_v, et[:], start=False, stop=True)
        s2 = ph2.tile([24, 512], F32, tag="s2")
        nc.vector.tensor_copy(s2[:], p2[:])
        ys = 8 * s + 3
        ya, yb = max(ys, 0), min(ys + 8, HW)
        while ya < yb:
            run = min(yb - ya, 128 - (ya % 128))
            p0 = ya % 128
            dst = g3[p0:p0 + run, :].rearrange(
                "p (s c x) -> p s c x", s=3, c=4)[:, :, ya // 128, :]
            sv = s2[(ya - ys) * 3:(ya - ys + run) * 3, :]
            nc.gpsimd.dma_start(dst, sv)
            ya += run

    # --- phases 2.5/3/4 as chunked functions, drip-fed into the wavefront
    asm_specs = [  # (ch, py, px, qp index or None, plane addend)
        (0, 0, 0, 0, 0), (0, 0, 1, None, 1),
        (0, 1, 0, 4, "g1"), (0, 1, 1, 3, 3),
        (1, 0, 0, None, 0), (1, 0, 1, None, "g0"),
        (1, 1, 0, None, "g1"), (1, 1, 1, None, 3),
        (2, 0, 0, 2, 0), (2, 0, 1, 5, "g0"),
        (2, 1, 0, None, 2), (2, 1, 1, 1, 3),
    ]
    qp6_h = {}

    def phase25(hc):
        # green + chroma for y rows 64*hc .. 64*hc+63
        t = hc // 2
        p0 = (hc % 2) * 64
        cs = slice(t * 512, (t + 1) * 512)
        pr = slice(p0, p0 + 64)
        rd = rden[pr, 0:512]
        nc.vector.reciprocal(rd, den[pr, cs])
        nc.vector.tensor_mul(g0p[pr, cs], g0n[pr, cs], rd)
        nc.vector.tensor_mul(g1p[pr, cs], g1n[pr, cs], rd)
        nc.vector.tensor_sub(c1p[pr, cs], mos_pl(1)[pr, cs], g0p[pr, cs])
        nc.vector.tensor_sub(c2p[pr, cs], mos_pl(2)[pr, cs], g1p[pr, cs])

    def phase3_strip(sq):
        h = sq // 16
        if h not in qp6_h:
            qp6_h[h] = qpp.tile([128, 6 * 1024], BF16, tag="qp6",
                                name=f"qp6_{h}")
        qp6 = qp6_h[h]
        b3 = pools["b3"].tile([64, 514], BF16, tag="b3", name="b3")
        wrs = [nc.gpsimd.memset(b3[0:64, 0:514:513], 0.0)]
        y0 = 16 * sq - 1
        if y0 < 0:
            wrs.append(nc.gpsimd.memset(b3[0:1, :], 0.0))
            wrs.append(nc.gpsimd.memset(b3[32:33, :], 0.0))
        if y0 + 18 > HW:
            wrs.append(nc.sync.dma_start(b3[17:18, :], zt[0:1, 0:514]))
            wrs.append(nc.sync.dma_start(b3[49:50, :], zt[0:1, 0:514]))
        for cc, src_plane in ((0, c1p), (1, c2p)):
            ya, yb = max(y0, 0), min(y0 + 18, HW)
            while ya < yb:
                run = min(yb - ya, 128 - (ya % 128))
                d = nc.sync.dma_start(
                    b3[cc * 32 + ya - y0:cc * 32 + ya - y0 + run, 1:513],
                    src_plane[ya % 128:ya % 128 + run,
                              (ya // 128) * 512:(ya // 128) * 512 + 512])
                wrs.append(d)
                ya += run
        wrs.append(nc.gpsimd.dma_start(b3[18:32, :], zt[0:14, 0:514]))
        wrs.append(nc.gpsimd.dma_start(b3[50:64, :], zt[0:14, 0:514]))
        p3 = ps3.tile([96, 512], F32, tag="p3", name="p3")
        mm3 = [nc.tensor.matmul(p3[:], w5_v(dx), b3[0:64, dx:dx + 512],
                                start=(dx == 0), stop=(dx == 2))
               for dx in range(3)]
        for mm in mm3:
            for wr in wrs:
                add_dep(mm, wr, reason="b3-ready")
        s3 = ph2.tile([96, 512], BF16, tag="s3")
        nc.scalar.copy(s3[:], p3[:])
        yq = 16 * sq
        tlc = (yq // 128) - 2 * h  # 0 or 1: 512-chunk within the half
        dst = qp6[yq % 128:yq % 128 + 16, :].rearrange(
            "p (o c x) -> p o c x", o=6, c=2)[:, :, tlc, :]
        nc.sync.dma_start(dst, s3[:])

    def asm_unit(t, ch, py):
        # one output plane [128 quad rows, 1024 cols] for plane-chunk t
        h, tl = t // 2, t % 2
        qp6 = qp6_h[h]
        a = asmp.tile([128, 1024], F32, tag="asm", name="asm")
        prev = None
        for (c_, py_, px, qo, addend) in asm_specs:
            if c_ != ch or py_ != py:
                continue
            if addend == "g0":
                ad = g0p[:, t * 512:(t + 1) * 512]
            elif addend == "g1":
                ad = g1p[:, t * 512:(t + 1) * 512]
            else:
                ad = mos_pl(addend)[:, t * 512:(t + 1) * 512]
            view = a[:].rearrange("p (x two) -> p two x", two=2)[:, px, :]
            if qo is None:
                w_ = nc.vector.tensor_copy(view, ad)
            else:
                w_ = nc.vector.tensor_add(
                    view,
                    qp6[:, qo * 1024 + tl * 512:qo * 1024 + tl * 512 + 512],
                    ad)
            if prev is not None:
                add_dep(w_, prev, reason="asm-interleave")
            prev = w_
        dst = out[ch].rearrange("(y two) x -> two y x", two=2)[
            py, t * 128:(t + 1) * 128, :]
        nc.sync.dma_start(dst, a[:])

    # ---------------- emission schedule
    # a FIFO of small background closures (phase25 pieces, phase3 strips,
    # assembly units) drip-fed into the wavefront: bounded per-iteration
    # engine load, so the PE never starves (HAM stays at full clock)
    import collections
    bg = collections.deque()  # entries: (ready_iteration, closure)
    done3 = -1
    emitted3 = set()
    cur_i = [0]

    def phase25_parts(hc):
        # reciprocal is split in 4 so no single DVE op blocks the queue
        t = hc // 2
        p0 = (hc % 2) * 64
        cs = slice(t * 512, (t + 1) * 512)
        pr = slice(p0, p0 + 64)
        rd = rden[pr, 0:512]

        def recip(j):
            def f():
                nc.vector.reciprocal(
                    rd[:, j * 128:(j + 1) * 128],
                    den[pr, t * 512 + j * 128:t * 512 + (j + 1) * 128])
            return f

        def part_b():
            nc.vector.tensor_mul(g0p[pr, cs], g0n[pr, cs], rd)
            nc.vector.tensor_sub(c1p[pr, cs], mos_pl(1)[pr, cs], g0p[pr, cs])

        def part_c():
            nc.vector.tensor_mul(g1p[pr, cs], g1n[pr, cs], rd)
            nc.vector.tensor_sub(c2p[pr, cs], mos_pl(2)[pr, cs], g1p[pr, cs])

        return [recip(0), recip(1), recip(2), recip(3), part_b, part_c]

    def note_phase3_done(sq):
        nonlocal done3
        emitted3.add(sq)
        while done3 + 1 in emitted3:
            done3 += 1
            if (done3 + 1) % 8 == 0:
                t = done3 // 8
                for ch in range(3):
                    for py in range(2):
                        bg.append((cur_i[0] + 1,
                                   lambda t=t, ch=ch, py=py: asm_unit(t, ch, py)))

    def unlock(hc):
        # after phase25(hc), chroma rows <= 64*hc+63 are valid; the +2 delay
        # keeps the gather DMAs from camping on the sync queue head while
        # phase25's DVE work is still draining
        nonlocal sq_next
        lim = 4 * hc + 2 if hc < 7 else 31
        while sq_next <= lim:
            def do3(sq=sq_next):
                phase3_strip(sq)
                note_phase3_done(sq)
            bg.append((cur_i[0] + 2, do3))
            sq_next += 1

    sq_next = 0
    load_b0(-1)

    for i in range(NSTRIP + 4):
        cur_i[0] = i
        s = i - 1  # L1 strip index
        if s + 1 <= 63:
            load_b0(s + 1)  # prefetch one iteration ahead
        if s <= 63:
            pt = conv_pair(b0_t[s][0], 40, lambda dx: w1_v(0, dx),
                           lambda dx: w1_v(1, dx), 0, deps=b0_t[s][1])
            evict_pair(pt, b1_t, s, "b1", 1)
            b0_t.pop(s - 1, None)
        # halo for b1[s-1] (consumed by L2 next iteration); for s == 64 the
        # source strip 64 doesn't exist -> zero fill via halo() fallback
        if s - 1 in b1_t:
            halo(b1_t, s - 1, nc.sync)
        t2 = s - 2  # L2 strip index
        if -1 <= t2 <= 63:
            bt = b1_t[t2][0]
            pt = conv_pair(bt, 120, lambda dx: w23_v(0, dx),
                           lambda dx: w23_v(1, dx), 514)
            evict_pair(pt, b2_t, t2, "b2", 2)
        if t2 - 1 in b2_t:
            halo(b2_t, t2 - 1, nc.gpsimd)
        t3 = s - 4  # L3 strip index
        if -1 <= t3 <= 63:
            bt = b2_t[t3][0]
            pt = conv_pair(bt, 120, lambda dx: w23_v(2, dx),
                           lambda dx: w23_v(3, dx), 514)
            phase2(t3, pt)
            b1_t.pop(t3, None)
            b2_t.pop(t3 - 1, None)
            if t3 >= 7 and (t3 - 7) % 8 == 0:
                hc = (t3 - 7) // 8
                bg.extend((i, f) for f in phase25_parts(hc))
                unlock(hc)
        # drip background closures (FIFO; head must be past its ready-iter)
        for _ in range(3):
            if bg and bg[0][0] <= i:
                bg.popleft()[1]()

    # drain whatever is left (tail) -- no budget, engines pipeline freely
    while bg:
        bg.popleft()[1]()

    if "dbgp" in outs:
        dbgp = outs["dbgp"]  # [4, 512, 512] bf16
        for j, pl in enumerate((g0p, g1p, c1p, c2p)):
            nc.sync.dma_start(
                dbgp[j].rearrange("(t p) x -> p t x", p=128),
                pl.rearrange("p (t x) -> p t x", t=4))
        dbgq = outs["dbgq"]  # [2, 128, 6144] bf16
        for j, h in enumerate(sorted(qp6_h)):
            nc.sync.dma_start(dbgq[j], qp6_h[h][:])


_CACHE = {}


def _get_compiled():
    if "nc" in _CACHE:
        return _CACHE["nc"]
    nc = bacc.Bacc("TRN2", target_bir_lowering=False, debug=False,
                   enable_asserts=False)
    ins = {
        "mospad": nc.dram_tensor("mospad", [4, 522, 514], BF16,
                                 kind="ExternalInput").ap(),
        "wblob": nc.dram_tensor("wblob", [120, WBLOB_COLS], BF16,
                                kind="ExternalInput").ap(),
    }
    outs = {"out": nc.dram_tensor("out", [3, 1024, 1024], F32,
                                  kind="ExternalOutput").ap()}
    from contextlib import ExitStack
    with tile.TileContext(nc) as tc, ExitStack() as ctx:
        build_kernel(tc, outs, ins, ctx)
    nc.compile()
    _CACHE["nc"] = nc
    return nc


def kernel(**inputs):
    nc = _get_compiled()
    mospad, shared = _host_prep(inputs)
    in_maps = []
    for b in range(8):
        m = {"mospad": np.ascontiguousarray(mospad[b])}
        m.update(shared)
        in_maps.append(m)
    res = run_bass_kernel_spmd(nc, in_maps, core_ids=list(range(8)))
    return np.stack([res.results[b]["out"] for b in range(8)])


# revision 23
# speedup vs baseline: 1.8538x; 1.8538x over previous
"""Trainium2 Bass kernel for the BasicQuadRGBV2 demosaic model.

Data-parallel over batch: 1 image per NeuronCore (8 cores).

Per-core dataflow (image [4,512,512] -> [3,1024,1024]):
  Phase 1  (conv stacks): two 3-layer CNNs (4->12->12->12, 3x3, relu) computed
           as block-banded bf16 matmuls. Layout: partitions = (y_row_window x
           chan), free dim = x. The y-taps of each 3x3 conv live inside a
           banded lhsT (contract over (y_in, c)); the x-taps are 3
           PSUM-accumulated matmuls over free-dim-shifted views. Strips of 8
           output rows; the output grid drifts +1 row per layer so PSUM
           evictions always land at natural partitions; strip-to-strip halo
           rows move via small DMAs emitted one iteration ahead. The f and w
           stacks share one strip tile (f in cols 0:514, w in 514:1028) so
           each halo is a single DMA.
  Phase 2  (softmax green): E=exp(relu-free w3), i=relu(f3); selector matmuls
           reduce over channels-in-partitions giving g0num/g1num/den planes
           (f32).
  Phase 2.5 (per 64-row half-chunk): rden~=1/den (fast approx); g0,g1 (bf16);
           chroma c1=mosaic1-g0, c2=mosaic2-g1 (bf16).
  Phase 3  (chroma 5x5 convs): in pixel-shuffled space each needed
           (conv, phase) output is a 12-tap stencil over (c1,c2) within a
           3x3 quad-space window -> same banded-matmul machinery, 6 outputs
           at once.
  Phase 4  (assembly): DVE/scalar writes with stride-2 free APs interleave
           quad planes into full-res f32 rows; contiguous row DMAs to DRAM.

Phases 2.5/3/4 are drip-fed into the wavefront via ready-queues (bounded
work per iteration per engine) so the PE never idles long enough for the
HAM clock-gate to drop it to half rate.
"""

import numpy as np
import ml_dtypes

import concourse.bass as bass
import concourse.tile as tile
from concourse import bacc, mybir
from concourse.tile import add_dep_helper as _adh


def add_dep(frm, to, reason=""):
    _adh(frm.ins, to.ins, reason=reason)


from concourse.bass_utils import run_bass_kernel_spmd

F32 = mybir.dt.float32
BF16 = mybir.dt.bfloat16
RELU = mybir.ActivationFunctionType.Relu
EXP = mybir.ActivationFunctionType.Exp
COPY = mybir.ActivationFunctionType.Copy

WIDTH = 12
HW = 512  # image H = W (quad space)
NSTRIP = 65  # strips s = -1 .. 63, stride 8

# wblob column offsets
W1_OFS = 0            # [40, 6*96]   (st,dx)
W23_OFS = 576         # [120, 12*96] (ly,dx)
SEL_OFS = W23_OFS + 1152   # [96, 48]
W5_OFS = SEL_OFS + 48      # [64, 3*96]
WBLOB_COLS = W5_OFS + 288


# ---------------------------------------------------------------- host prep

def _band_lhsT(W, cin):
    """W: [12, cin, 3, 3] -> [3, 10*cin, 96] banded matrices (one per x-tap).

    lhsT_dx[(yi*cin + c), (yo*12 + oc)] = W[oc, c, yi - yo, dx]
    """
    K, M = 10 * cin, 8 * WIDTH
    out = np.zeros((3, K, M), np.float32)
    for dx in range(3):
        for yo in range(8):
            for dy in range(3):
                yi = yo + dy
                out[dx, yi * cin:(yi + 1) * cin, yo * WIDTH:(yo + 1) * WIDTH] = \
                    W[:, :, dy, dx].T
    return out


def _selectors():
    selA = np.zeros((96, 24), np.float32)  # applied to i*E
    selB = np.zeros((96, 24), np.float32)  # applied to E
    for yl in range(8):
        for c in range(WIDTH):
            p = yl * WIDTH + c
            if c < 6:
                selA[p, yl * 3 + 0] = 1.0
            else:
                selA[p, yl * 3 + 1] = 1.0
            selB[p, yl * 3 + 2] = 1.0
    return selA, selB


def _g_stencil(K5, py, px):
    """12-tap quad-space stencil of a 5x5 conv output at phase (py,px),
    over chroma channels c1 (phase (0,1)) and c2 (phase (1,0))."""
    G = np.zeros((2, 3, 3), np.float32)
    for cc, (qy, qx) in enumerate(((0, 1), (1, 0))):
        for dy in (-1, 0, 1):
            for dx in (-1, 0, 1):
                d5y = 2 * dy + 2 - py + qy
                d5x = 2 * dx + 2 - px + qx
                if 0 <= d5y < 5 and 0 <= d5x < 5:
                    G[cc, dy + 1, dx + 1] = K5[d5y, d5x]
    return G


def _chroma_lhsT(chw, cvw, cqw):
    """-> [3, 64, 96] banded matrices for the 6 (conv, phase) outputs.

    Output order o: 0 ch@(0,0), 1 ch@(1,1), 2 cv@(0,0), 3 cv@(1,1),
                    4 cq@(1,0), 5 cq@(0,1).
    """
    specs = [(chw, 0, 0), (chw, 1, 1), (cvw, 0, 0), (cvw, 1, 1),
             (cqw, 1, 0), (cqw, 0, 1)]
    out = np.zeros((3, 64, 96), np.float32)
    for o, (K5, py, px) in enumerate(specs):
        G = _g_stencil(np.asarray(K5)[0, 0], py, px)
        for dx in range(3):
            for yo in range(16):
                for dy in (-1, 0, 1):
                    yi = yo + dy + 1
                    for cc in range(2):
                        out[dx, cc * 32 + yi, yo * 6 + o] = G[cc, dy + 1, dx]
    return out


def _host_prep(inputs):
    mosaic = np.asarray(inputs["mosaic"], np.float32)  # [8,4,512,512]
    mospad = np.zeros((mosaic.shape[0], 4, 522, 514), ml_dtypes.bfloat16)
    mospad[:, :, 8:520, 1:513] = mosaic
    wblob = np.zeros((120, WBLOB_COLS), np.float32)
    w1 = [_band_lhsT(np.asarray(inputs["fw0"]), 4),
          _band_lhsT(np.asarray(inputs["ww0"]), 4)]
    for st in range(2):
        for dx in range(3):
            wblob[0:40, W1_OFS + (st * 3 + dx) * 96:
                  W1_OFS + (st * 3 + dx + 1) * 96] = w1[st][dx]
    w23 = [_band_lhsT(np.asarray(inputs["fw1"]), 12),
           _band_lhsT(np.asarray(inputs["ww1"]), 12),
           _band_lhsT(np.asarray(inputs["fw2"]), 12),
           _band_lhsT(np.asarray(inputs["ww2"]), 12)]
    for ly in range(4):
        for dx in range(3):
            wblob[0:120, W23_OFS + (ly * 3 + dx) * 96:
                  W23_OFS + (ly * 3 + dx + 1) * 96] = w23[ly][dx]
    selA, selB = _selectors()
    wblob[0:96, SEL_OFS:SEL_OFS + 24] = selA
    wblob[0:96, SEL_OFS + 24:SEL_OFS + 48] = selB
    w5 = _chroma_lhsT(inputs["chw"], inputs["cvw"], inputs["cqw"])
    for dx in range(3):
        wblob[0:64, W5_OFS + dx * 96:W5_OFS + (dx + 1) * 96] = w5[dx]
    return mospad, {"wblob": wblob.astype(ml_dtypes.bfloat16)}


# ---------------------------------------------------------------- kernel IR

def build_kernel(tc, outs, ins, ctx):
    nc = tc.nc
    mospad, wblob = ins["mospad"], ins["wblob"]
    out = outs["out"]

    wp = ctx.enter_context(tc.tile_pool(name="weights", bufs=1))
    pp = ctx.enter_context(tc.tile_pool(name="planes", bufs=1))
    # conv layer pairs accumulate f|w into one 2-bank PSUM tile -> single
    # paired eviction ACTIVATE; 3 pairs (L1,L2,L3) + p2 + p3 = 8 banks
    psp = ctx.enter_context(tc.tile_pool(name="psp", bufs=3, space="PSUM"))
    ps2 = ctx.enter_context(tc.tile_pool(name="ps2", bufs=1, space="PSUM"))
    ps3 = ctx.enter_context(tc.tile_pool(name="ps3", bufs=1, space="PSUM"))
    pools = {}
    for tag, b in (("b0", 4), ("b1", 4), ("b2", 4), ("b3", 4)):
        pools[tag] = ctx.enter_context(tc.tile_pool(name=f"p_{tag}", bufs=b))
    ph2 = ctx.enter_context(tc.tile_pool(name="ph2", bufs=3))
    qpp = ctx.enter_context(tc.tile_pool(name="qp", bufs=2))
    asmp = ctx.enter_context(tc.tile_pool(name="asm", bufs=4))

    # --- weights to SBUF (single DMA)
    wb = wp.tile([120, WBLOB_COLS], BF16, tag="wb")
    nc.sync.dma_start(wb[:], wblob)

    def w1_v(st, dx):
        return wb[0:40, W1_OFS + (st * 3 + dx) * 96:W1_OFS + (st * 3 + dx + 1) * 96]

    def w23_v(ly, dx):
        return wb[0:120, W23_OFS + (ly * 3 + dx) * 96:
                  W23_OFS + (ly * 3 + dx + 1) * 96]

    selA_v = wb[0:96, SEL_OFS:SEL_OFS + 24]
    selB_v = wb[0:96, SEL_OFS + 24:SEL_OFS + 48]

    def w5_v(dx):
        return wb[0:64, W5_OFS + dx * 96:W5_OFS + (dx + 1) * 96]

    # --- persistent planes [128, 2048]: y -> (y%128, (y//128)*512 + x)
    mosp = pp.tile([128, 4 * 2048], BF16, tag="mosp")
    for c in range(4):
        nc.sync.dma_start(
            mosp[:, c * 2048:(c + 1) * 2048].rearrange(
                "p (t x) -> p t x", t=4),
            mospad[c, 8:520, 1:513].rearrange("(t p) x -> p t x", p=128))

    def mos_pl(c):
        return mosp[:, c * 2048:(c + 1) * 2048]

    g3 = pp.tile([128, 3 * 2048], F32, tag="g3")
    g0n = g3[:, 0:2048]
    g1n = g3[:, 2048:4096]
    den = g3[:, 4096:6144]
    g01 = pp.tile([128, 2 * 2048], BF16, tag="g01")
    g0p = g01[:, 0:2048]
    g1p = g01[:, 2048:4096]
    c12 = pp.tile([128, 2 * 2048], BF16, tag="c12")
    c1p = c12[:, 0:2048]
    c2p = c12[:, 2048:4096]
    rden = pp.tile([128, 512], F32, tag="rden")
    zt = pp.tile([96, 1028], BF16, tag="zt")
    nc.gpsimd.memset(zt[:], 0.0)

    # --- phase 1+2 wavefront over strips
    b0_t, b1_t, b2_t = {}, {}, {}

    def load_b0(s):
        t = pools["b0"].tile([40, 514], BF16, tag="b0", name="b0")
        src = mospad[:, 8 * s + 8:8 * s + 18, :].transpose([1, 0, 2])
        d = nc.sync.dma_start(t[:], src)
        b0_t[s] = (t, [d])

    def conv_pair(rhs_tile, kdim, wvf, wvw, cofs_w, deps=()):
        # f stack -> psum cols 0:512, w stack -> 512:1024 (adjacent banks)
        pt = psp.tile([96, 1024], F32, tag="cp2", name="cp2")
        mms = []
        for half, (wv, cofs) in enumerate(((wvf, 0), (wvw, cofs_w))):
            for dx in range(3):
                mms.append(nc.tensor.matmul(
                    pt[:, half * 512:half * 512 + 512], wv(dx),
                    rhs_tile[0:kdim, cofs + dx:cofs + dx + 512],
                    start=(dx == 0), stop=(dx == 2)))
        for mm in mms:
            for dep in deps:
                add_dep(mm, dep, reason="rhs-ready")
        return pt

    def evict_pair(pt, store, s, tag, k):
        # strip rows m=0..7 hold y = 8s+k+m; rows outside [0,512) must be
        # exactly zero (conv zero-padding) or they leak into the next layer.
        # f stack -> cols 1:513, w stack -> 515:1027 of one tile, written by
        # a single two-segment ACTIVATE from the paired PSUM.
        t = pools[tag].tile([120, 1028], BF16, tag=tag, name=tag)
        a1 = nc.scalar.activation(
            t[0:96, :].rearrange("p (h x) -> p h x", h=2)[:, :, 1:513],
            pt[:], RELU)
        z0 = nc.gpsimd.memset(
            t[0:96, :].rearrange("p (h x) -> p h x", h=2)[:, :, 0:514:513], 0.0)
        add_dep(z0, a1, reason="pad-cols")
        insts = [a1, z0]
        if s == -1 and 8 - k > 0:
            z = nc.sync.dma_start(t[0:(8 - k) * 12, :], zt[0:(8 - k) * 12, :])
            for a in (a1, z0):
                add_dep(z, a, reason="zero-pad-rows")
            insts.append(z)
        if s == 63 and 8 - k < 8:
            z = nc.sync.dma_start(t[(8 - k) * 12:96, :], zt[0:k * 12, :])
            for a in (a1, z0):
                add_dep(z, a, reason="zero-pad-rows")
            insts.append(z)
        store[s] = (t, insts)

    def halo(store, s, eng):
        # store[s][96:120] <- store[s+1][0:24]  (rows y+8, y+9)
        dst, insts = store[s]
        if s + 1 in store:
            d = eng.dma_start(dst[96:120, :], store[s + 1][0][0:24, :])
        else:
            d = eng.dma_start(dst[96:120, :], zt[0:24, :])
        for i_ in insts:
            add_dep(d, i_, reason="halo-after-evict")

    def phase2(s, pt):
        it = ph2.tile([96, 512], BF16, tag="i")
        et = ph2.tile([96, 512], BF16, tag="e")
        nc.vector.tensor_scalar_max(it[:], pt[:, 0:512], 0.0)
        nc.scalar.activation(et[:], pt[:, 512:1024], EXP)
        # exp(relu(x)) == max(exp(x), 1) -- this IS the last-layer relu
        # (gpsimd tensor ops are ~15x slower than DVE -- keep on vector)
        nc.vector.tensor_scalar_max(et[:], et[:], 1.0)
        nc.vector.tensor_mul(it[:], it[:], et[:])  # i*E in place
        p2 = ps2.tile([24, 512], F32, tag="p2")
        nc.tensor.matmul(p2[:], selA_v, it[:], start=True, stop=False)
        nc.tensor.matmul(p2[:], selB_v, et[:], start=False, stop=True)
        s2 = ph2.tile([24, 512], F32, tag="s2")
        nc.vector.tensor_copy(s2[:], p2[:])
        ys = 8 * s + 3
        ya, yb = max(ys, 0), min(ys + 8, HW)
        while ya < yb:
            run = min(yb - ya, 128 - (ya % 128))
            p0 = ya % 128
            dst = g3[p0:p0 + run, :].rearrange(
                "p (s c x) -> p s c x", s=3, c=4)[:, :, ya // 128, :]
            sv = s2[(ya - ys) * 3:(ya - ys + run) * 3, :]
            nc.gpsimd.dma_start(dst, sv)
            ya += run

    # --- phases 2.5/3/4 as chunked functions, drip-fed into the wavefront
    asm_specs = [  # (ch, py, px, qp index or None, plane addend)
        (0, 0, 0, 0, 0), (0, 0, 1, None, 1),
        (0, 1, 0, 4, "g1"), (0, 1, 1, 3, 3),
        (1, 0, 0, None, 0), (1, 0, 1, None, "g0"),
        (1, 1, 0, None, "g1"), (1, 1, 1, None, 3),
        (2, 0, 0, 2, 0), (2, 0, 1, 5, "g0"),
        (2, 1, 0, None, 2), (2, 1, 1, 1, 3),
    ]
    qp6_h = {}

    def phase25(hc):
        # green + chroma for y rows 64*hc .. 64*hc+63
        t = hc // 2
        p0 = (hc % 2) * 64
        cs = slice(t * 512, (t + 1) * 512)
        pr = slice(p0, p0 + 64)
        rd = rden[pr, 0:512]
        nc.vector.reciprocal(rd, den[pr, cs])
        nc.vector.tensor_mul(g0p[pr, cs], g0n[pr, cs], rd)
        nc.vector.tensor_mul(g1p[pr, cs], g1n[pr, cs], rd)
        nc.vector.tensor_sub(c1p[pr, cs], mos_pl(1)[pr, cs], g0p[pr, cs])
        nc.vector.tensor_sub(c2p[pr, cs], mos_pl(2)[pr, cs], g1p[pr, cs])

    def phase3_strip(sq):
        h = sq // 16
        if h not in qp6_h:
            qp6_h[h] = qpp.tile([128, 6 * 1024], BF16, tag="qp6",
                                name=f"qp6_{h}")
        qp6 = qp6_h[h]
        b3 = pools["b3"].tile([64, 514], BF16, tag="b3", name="b3")
        wrs = [nc.gpsimd.memset(b3[0:64, 0:514:513], 0.0)]
        y0 = 16 * sq - 1
        if y0 < 0:
            wrs.append(nc.gpsimd.memset(b3[0:1, :], 0.0))
            wrs.append(nc.gpsimd.memset(b3[32:33, :], 0.0))
        if y0 + 18 > HW:
            wrs.append(nc.sync.dma_start(b3[17:18, :], zt[0:1, 0:514]))
            wrs.append(nc.sync.dma_start(b3[49:50, :], zt[0:1, 0:514]))
        for cc, src_plane in ((0, c1p), (1, c2p)):
            ya, yb = max(y0, 0), min(y0 + 18, HW)
            while ya < yb:
                run = min(yb - ya, 128 - (ya % 128))
                d = nc.sync.dma_start(
                    b3[cc * 32 + ya - y0:cc * 32 + ya - y0 + run, 1:513],
                    src_plane[ya % 128:ya % 128 + run,
                              (ya // 128) * 512:(ya // 128) * 512 + 512])
                wrs.append(d)
                ya += run
        wrs.append(nc.gpsimd.dma_start(b3[18:32, :], zt[0:14, 0:514]))
        wrs.append(nc.gpsimd.dma_start(b3[50:64, :], zt[0:14, 0:514]))
        p3 = ps3.tile([96, 512], F32, tag="p3", name="p3")
        mm3 = [nc.tensor.matmul(p3[:], w5_v(dx), b3[0:64, dx:dx + 512],
                                start=(dx == 0), stop=(dx == 2))
               for dx in range(3)]
        for mm in mm3:
            for wr in wrs:
                add_dep(mm, wr, reason="b3-ready")
        s3 = ph2.tile([96, 512], BF16, tag="s3")
        nc.scalar.copy(s3[:], p3[:])
        yq = 16 * sq
        tlc = (yq // 128) - 2 * h  # 0 or 1: 512-chunk within the half
        dst = qp6[yq % 128:yq % 128 + 16, :].rearrange(
            "p (o c x) -> p o c x", o=6, c=2)[:, :, tlc, :]
        nc.sync.dma_start(dst, s3[:])

    def asm_unit(t, ch, py):
        # one output plane [128 quad rows, 1024 cols] for plane-chunk t
        h, tl = t // 2, t % 2
        qp6 = qp6_h[h]
        a = asmp.tile([128, 1024], F32, tag="asm", name="asm")
        prev = None
        for (c_, py_, px, qo, addend) in asm_specs:
            if c_ != ch or py_ != py:
                continue
            if addend == "g0":
                ad = g0p[:, t * 512:(t + 1) * 512]
            elif addend == "g1":
                ad = g1p[:, t * 512:(t + 1) * 512]
            else:
                ad = mos_pl(addend)[:, t * 512:(t + 1) * 512]
            view = a[:].rearrange("p (x two) -> p two x", two=2)[:, px, :]
            if qo is None:
                w_ = nc.vector.tensor_copy(view, ad)
            else:
                w_ = nc.vector.tensor_add(
                    view,
                    qp6[:, qo * 1024 + tl * 512:qo * 1024 + tl * 512 + 512],
                    ad)
            if prev is not None:
                add_dep(w_, prev, reason="asm-interleave")
            prev = w_
        dst = out[ch].rearrange("(y two) x -> two y x", two=2)[
            py, t * 128:(t + 1) * 128, :]
        nc.sync.dma_start(dst, a[:])

    # ---------------- emission schedule
    # a FIFO of small background closures (phase25 pieces, phase3 strips,
    # assembly units) drip-fed into the wavefront: bounded per-iteration
    # engine load, so the PE never starves (HAM stays at full clock)
    import collections
    bg = collections.deque()  # entries: (ready_iteration, closure)
    done3 = -1
    emitted3 = set()
    cur_i = [0]

    def phase25_parts(hc):
        # reciprocal is split in 4 so no single DVE op blocks the queue
        t = hc // 2
        p0 = (hc % 2) * 64
        cs = slice(t * 512, (t + 1) * 512)
        pr = slice(p0, p0 + 64)
        rd = rden[pr, 0:512]

        def recip(j):
            def f():
                nc.vector.reciprocal(
                    rd[:, j * 128:(j + 1) * 128],
                    den[pr, t * 512 + j * 128:t * 512 + (j + 1) * 128])
            return f

        def part_b():
            nc.vector.tensor_mul(g0p[pr, cs], g0n[pr, cs], rd)
            nc.vector.tensor_sub(c1p[pr, cs], mos_pl(1)[pr, cs], g0p[pr, cs])

        def part_c():
            nc.vector.tensor_mul(g1p[pr, cs], g1n[pr, cs], rd)
            nc.vector.tensor_sub(c2p[pr, cs], mos_pl(2)[pr, cs], g1p[pr, cs])

        return [recip(0), recip(1), recip(2), recip(3), part_b, part_c]

    def note_phase3_done(sq):
        nonlocal done3
        emitted3.add(sq)
        while done3 + 1 in emitted3:
            done3 += 1
            if (done3 + 1) % 8 == 0:
                t = done3 // 8
                for ch in range(3):
                    for py in range(2):
                        bg.append((cur_i[0] + 1,
                                   lambda t=t, ch=ch, py=py: asm_unit(t, ch, py)))

    def unlock(hc):
        # after phase25(hc), chroma rows <= 64*hc+63 are valid; the +2 delay
        # keeps the gather DMAs from camping on the sync queue head while
        # phase25's DVE work is still draining
        nonlocal sq_next
        lim = 4 * hc + 2 if hc < 7 else 31
        while sq_next <= lim:
            def do3(sq=sq_next):
                phase3_strip(sq)
                note_phase3_done(sq)
            bg.append((cur_i[0] + 2, do3))
            sq_next += 1

    sq_next = 0
    load_b0(-1)

    for i in range(NSTRIP + 4):
        cur_i[0] = i
        s = i - 1  # L1 strip index
        if s + 1 <= 63:
            load_b0(s + 1)  # prefetch one iteration ahead
        if s <= 63:
            pt = conv_pair(b0_t[s][0], 40, lambda dx: w1_v(0, dx),
                           lambda dx: w1_v(1, dx), 0, deps=b0_t[s][1])
            evict_pair(pt, b1_t, s, "b1", 1)
            b0_t.pop(s - 1, None)
        # halo for b1[s-1] (consumed by L2 next iteration); for s == 64 the
        # source strip 64 doesn't exist -> zero fill via halo() fallback
        if s - 1 in b1_t:
            halo(b1_t, s - 1, nc.sync)
        t2 = s - 2  # L2 strip index
        if -1 <= t2 <= 63:
            bt = b1_t[t2][0]
            pt = conv_pair(bt, 120, lambda dx: w23_v(0, dx),
                           lambda dx: w23_v(1, dx), 514)
            evict_pair(pt, b2_t, t2, "b2", 2)
        if t2 - 1 in b2_t:
            halo(b2_t, t2 - 1, nc.gpsimd)
        t3 = s - 4  # L3 strip index
        if -1 <= t3 <= 63:
            bt = b2_t[t3][0]
            pt = conv_pair(bt, 120, lambda dx: w23_v(2, dx),
                           lambda dx: w23_v(3, dx), 514)
            phase2(t3, pt)
            b1_t.pop(t3, None)
            b2_t.pop(t3 - 1, None)
            if t3 >= 7 and (t3 - 7) % 8 == 0:
                hc = (t3 - 7) // 8
                bg.extend((i, f) for f in phase25_parts(hc))
                unlock(hc)
        # drip background closures (FIFO; head must be past its ready-iter)
        for _ in range(3):
            if bg and bg[0][0] <= i:
                bg.popleft()[1]()

    # drain whatever is left (tail) -- no budget, engines pipeline freely
    while bg:
        bg.popleft()[1]()

    if "dbgp" in outs:
        dbgp = outs["dbgp"]  # [4, 512, 512] bf16
        for j, pl in enumerate((g0p, g1p, c1p, c2p)):
            nc.sync.dma_start(
                dbgp[j].rearrange("(t p) x -> p t x", p=128),
                pl.rearrange("p (t x) -> p t x", t=4))
        dbgq = outs["dbgq"]  # [2, 128, 6144] bf16
        for j, h in enumerate(sorted(qp6_h)):
            nc.sync.dma_start(dbgq[j], qp6_h[h][:])


_CACHE = {}


def _get_compiled():
    if "nc" in _CACHE:
        return _CACHE["nc"]
    nc = bacc.Bacc("TRN2", target_bir_lowering=False, debug=False,
                   enable_asserts=False)
    ins = {
        "mospad": nc.dram_tensor("mospad", [4, 522, 514], BF16,
                                 kind="ExternalInput").ap(),
        "wblob": nc.dram_tensor("wblob", [120, WBLOB_COLS], BF16,
                                kind="ExternalInput").ap(),
    }
    outs = {"out": nc.dram_tensor("out", [3, 1024, 1024], F32,
                                  kind="ExternalOutput").ap()}
    from contextlib import ExitStack
    with tile.TileContext(nc) as tc, ExitStack() as ctx:
        build_kernel(tc, outs, ins, ctx)
    nc.compile()
    _CACHE["nc"] = nc
    return nc


def kernel(**inputs):
    nc = _get_compiled()
    mospad, shared = _host_prep(inputs)
    in_maps = []
    for b in range(8):
        m = {"mospad": np.ascontiguousarray(mospad[b])}
        m.update(shared)
        in_maps.append(m)
    res = run_bass_kernel_spmd(nc, in_maps, core_ids=list(range(8)))
    return np.stack([res.results[b]["out"] for b in range(8)])


# revision 25
# speedup vs baseline: 1.9578x; 1.0561x over previous
"""Trainium2 Bass kernel for the BasicQuadRGBV2 demosaic model.

Data-parallel over batch: 1 image per NeuronCore (8 cores).

Per-core dataflow (image [4,512,512] -> [3,1024,1024]):
  Phase 1  (conv stacks): two 3-layer CNNs (4->12->12->12, 3x3, relu) computed
           as block-banded bf16 matmuls. Layout: partitions = (y_row_window x
           chan), free dim = x. The y-taps of each 3x3 conv live inside a
           banded lhsT (contract over (y_in, c)); the x-taps are 3
           PSUM-accumulated matmuls over free-dim-shifted views. Strips of 8
           output rows; the output grid drifts +1 row per layer so PSUM
           evictions always land at natural partitions; strip-to-strip halo
           rows move via small DMAs emitted one iteration ahead. The f and w
           stacks share one strip tile (f in cols 0:514, w in 514:1028) so
           each halo is a single DMA.
  Phase 2  (softmax green): E=exp(relu-free w3), i=relu(f3); selector matmuls
           reduce over channels-in-partitions giving g0num/g1num/den planes
           (f32).
  Phase 2.5 (per 64-row half-chunk): rden~=1/den (fast approx); g0,g1 (bf16);
           chroma c1=mosaic1-g0, c2=mosaic2-g1 (bf16).
  Phase 3  (chroma 5x5 convs): in pixel-shuffled space each needed
           (conv, phase) output is a 12-tap stencil over (c1,c2) within a
           3x3 quad-space window -> same banded-matmul machinery, 6 outputs
           at once.
  Phase 4  (assembly): DVE/scalar writes with stride-2 free APs interleave
           quad planes into full-res f32 rows; contiguous row DMAs to DRAM.

Phases 2.5/3/4 are drip-fed into the wavefront via ready-queues (bounded
work per iteration per engine) so the PE never idles long enough for the
HAM clock-gate to drop it to half rate.
"""

import numpy as np
import ml_dtypes

import concourse.bass as bass
import concourse.tile as tile
from concourse import bacc, mybir
from concourse.tile import add_dep_helper as _adh


def add_dep(frm, to, reason=""):
    _adh(frm.ins, to.ins, reason=reason)


from concourse.bass_utils import run_bass_kernel_spmd

F32 = mybir.dt.float32
BF16 = mybir.dt.bfloat16
RELU = mybir.ActivationFunctionType.Relu
EXP = mybir.ActivationFunctionType.Exp
COPY = mybir.ActivationFunctionType.Copy

WIDTH = 12
HW = 512  # image H = W (quad space)
NSTRIP = 65  # strips s = -1 .. 63, stride 8

# wblob column offsets
W1_OFS = 0            # [40, 6*96]   (st,dx)
W23_OFS = 576         # [120, 12*96] (ly,dx)
SEL_OFS = W23_OFS + 1152   # [96, 48]
W5_OFS = SEL_OFS + 48      # [64, 3*96]
WBLOB_COLS = W5_OFS + 288


# ---------------------------------------------------------------- host prep

def _band_lhsT(W, cin):
    """W: [12, cin, 3, 3] -> [3, 10*cin, 96] banded matrices (one per x-tap).

    lhsT_dx[(yi*cin + c), (yo*12 + oc)] = W[oc, c, yi - yo, dx]
    """
    K, M = 10 * cin, 8 * WIDTH
    out = np.zeros((3, K, M), np.float32)
    for dx in range(3):
        for yo in range(8):
            for dy in range(3):
                yi = yo + dy
                out[dx, yi * cin:(yi + 1) * cin, yo * WIDTH:(yo + 1) * WIDTH] = \
                    W[:, :, dy, dx].T
    return out


def _selectors():
    selA = np.zeros((96, 24), np.float32)  # applied to i*E
    selB = np.zeros((96, 24), np.float32)  # applied to E
    for yl in range(8):
        for c in range(WIDTH):
            p = yl * WIDTH + c
            if c < 6:
                selA[p, yl * 3 + 0] = 1.0
            else:
                selA[p, yl * 3 + 1] = 1.0
            selB[p, yl * 3 + 2] = 1.0
    return selA, selB


def _g_stencil(K5, py, px):
    """12-tap quad-space stencil of a 5x5 conv output at phase (py,px),
    over chroma channels c1 (phase (0,1)) and c2 (phase (1,0))."""
    G = np.zeros((2, 3, 3), np.float32)
    for cc, (qy, qx) in enumerate(((0, 1), (1, 0))):
        for dy in (-1, 0, 1):
            for dx in (-1, 0, 1):
                d5y = 2 * dy + 2 - py + qy
                d5x = 2 * dx + 2 - px + qx
                if 0 <= d5y < 5 and 0 <= d5x < 5:
                    G[cc, dy + 1, dx + 1] = K5[d5y, d5x]
    return G


def _chroma_lhsT(chw, cvw, cqw):
    """-> [3, 64, 96] banded matrices for the 6 (conv, phase) outputs.

    Output order o: 0 ch@(0,0), 1 ch@(1,1), 2 cv@(0,0), 3 cv@(1,1),
                    4 cq@(1,0), 5 cq@(0,1).
    """
    specs = [(chw, 0, 0), (chw, 1, 1), (cvw, 0, 0), (cvw, 1, 1),
             (cqw, 1, 0), (cqw, 0, 1)]
    out = np.zeros((3, 64, 96), np.float32)
    for o, (K5, py, px) in enumerate(specs):
        G = _g_stencil(np.asarray(K5)[0, 0], py, px)
        for dx in range(3):
            for yo in range(16):
                for dy in (-1, 0, 1):
                    yi = yo + dy + 1
                    for cc in range(2):
                        out[dx, cc * 32 + yi, yo * 6 + o] = G[cc, dy + 1, dx]
    return out


def _host_prep(inputs):
    mosaic = np.asarray(inputs["mosaic"], np.float32)  # [8,4,512,512]
    mospad = np.zeros((mosaic.shape[0], 4, 522, 514), ml_dtypes.bfloat16)
    mospad[:, :, 8:520, 1:513] = mosaic
    wblob = np.zeros((120, WBLOB_COLS), np.float32)
    w1 = [_band_lhsT(np.asarray(inputs["fw0"]), 4),
          _band_lhsT(np.asarray(inputs["ww0"]), 4)]
    for st in range(2):
        for dx in range(3):
            wblob[0:40, W1_OFS + (st * 3 + dx) * 96:
                  W1_OFS + (st * 3 + dx + 1) * 96] = w1[st][dx]
    w23 = [_band_lhsT(np.asarray(inputs["fw1"]), 12),
           _band_lhsT(np.asarray(inputs["ww1"]), 12),
           _band_lhsT(np.asarray(inputs["fw2"]), 12),
           _band_lhsT(np.asarray(inputs["ww2"]), 12)]
    for ly in range(4):
        for dx in range(3):
            wblob[0:120, W23_OFS + (ly * 3 + dx) * 96:
                  W23_OFS + (ly * 3 + dx + 1) * 96] = w23[ly][dx]
    selA, selB = _selectors()
    wblob[0:96, SEL_OFS:SEL_OFS + 24] = selA
    wblob[0:96, SEL_OFS + 24:SEL_OFS + 48] = selB
    w5 = _chroma_lhsT(inputs["chw"], inputs["cvw"], inputs["cqw"])
    for dx in range(3):
        wblob[0:64, W5_OFS + dx * 96:W5_OFS + (dx + 1) * 96] = w5[dx]
    return mospad, {"wblob": wblob.astype(ml_dtypes.bfloat16)}


# ---------------------------------------------------------------- kernel IR

def build_kernel(tc, outs, ins, ctx):
    nc = tc.nc
    mospad, wblob = ins["mospad"], ins["wblob"]
    out = outs["out"]

    wp = ctx.enter_context(tc.tile_pool(name="weights", bufs=1))
    pp = ctx.enter_context(tc.tile_pool(name="planes", bufs=1))
    # conv layer pairs accumulate f|w into one 2-bank PSUM tile -> single
    # paired eviction ACTIVATE; 3 pairs (L1,L2,L3) + p2 + p3 = 8 banks
    psp = ctx.enter_context(tc.tile_pool(name="psp", bufs=3, space="PSUM"))
    ps2 = ctx.enter_context(tc.tile_pool(name="ps2", bufs=1, space="PSUM"))
    ps3 = ctx.enter_context(tc.tile_pool(name="ps3", bufs=1, space="PSUM"))
    pools = {}
    for tag, b in (("b0", 4), ("b1", 4), ("b2", 4), ("b3", 4)):
        pools[tag] = ctx.enter_context(tc.tile_pool(name=f"p_{tag}", bufs=b))
    ph2 = ctx.enter_context(tc.tile_pool(name="ph2", bufs=3))
    qpp = ctx.enter_context(tc.tile_pool(name="qp", bufs=2))
    asmp = ctx.enter_context(tc.tile_pool(name="asm", bufs=4))

    # --- weights to SBUF (single DMA)
    wb = wp.tile([120, WBLOB_COLS], BF16, tag="wb")
    nc.sync.dma_start(wb[:], wblob)

    def w1_v(st, dx):
        return wb[0:40, W1_OFS + (st * 3 + dx) * 96:W1_OFS + (st * 3 + dx + 1) * 96]

    def w23_v(ly, dx):
        return wb[0:120, W23_OFS + (ly * 3 + dx) * 96:
                  W23_OFS + (ly * 3 + dx + 1) * 96]

    selA_v = wb[0:96, SEL_OFS:SEL_OFS + 24]
    selB_v = wb[0:96, SEL_OFS + 24:SEL_OFS + 48]

    def w5_v(dx):
        return wb[0:64, W5_OFS + dx * 96:W5_OFS + (dx + 1) * 96]

    # --- persistent planes [128, 2048]: y -> (y%128, (y//128)*512 + x)
    mosp = pp.tile([128, 4 * 2048], BF16, tag="mosp")
    for c in range(4):
        nc.sync.dma_start(
            mosp[:, c * 2048:(c + 1) * 2048].rearrange(
                "p (t x) -> p t x", t=4),
            mospad[c, 8:520, 1:513].rearrange("(t p) x -> p t x", p=128))

    def mos_pl(c):
        return mosp[:, c * 2048:(c + 1) * 2048]

    g3 = pp.tile([128, 3 * 2048], F32, tag="g3")
    g0n = g3[:, 0:2048]
    g1n = g3[:, 2048:4096]
    den = g3[:, 4096:6144]
    g01 = pp.tile([128, 2 * 2048], BF16, tag="g01")
    g0p = g01[:, 0:2048]
    g1p = g01[:, 2048:4096]
    c12 = pp.tile([128, 2 * 2048], BF16, tag="c12")
    c1p = c12[:, 0:2048]
    c2p = c12[:, 2048:4096]
    rden = pp.tile([128, 512], F32, tag="rden")
    zt = pp.tile([96, 1028], BF16, tag="zt")
    nc.gpsimd.memset(zt[:], 0.0)

    # --- phase 1+2 wavefront over strips
    b0_t, b1_t, b2_t = {}, {}, {}

    def load_b0(s):
        t = pools["b0"].tile([40, 514], BF16, tag="b0", name="b0")
        src = mospad[:, 8 * s + 8:8 * s + 18, :].transpose([1, 0, 2])
        d = nc.sync.dma_start(t[:], src)
        b0_t[s] = (t, [d])

    def conv_pair(rhs_tile, kdim, wvf, wvw, cofs_w, deps=()):
        # f stack -> psum cols 0:512, w stack -> 512:1024 (adjacent banks)
        pt = psp.tile([96, 1024], F32, tag="cp2", name="cp2")
        mms = []
        for half, (wv, cofs) in enumerate(((wvf, 0), (wvw, cofs_w))):
            for dx in range(3):
                mms.append(nc.tensor.matmul(
                    pt[:, half * 512:half * 512 + 512], wv(dx),
                    rhs_tile[0:kdim, cofs + dx:cofs + dx + 512],
                    start=(dx == 0), stop=(dx == 2)))
        for mm in mms:
            for dep in deps:
                add_dep(mm, dep, reason="rhs-ready")
        return pt

    def evict_pair(pt, store, s, tag, k):
        # strip rows m=0..7 hold y = 8s+k+m; rows outside [0,512) must be
        # exactly zero (conv zero-padding) or they leak into the next layer.
        # f stack -> cols 1:513, w stack -> 515:1027 of one tile, written by
        # a single two-segment ACTIVATE from the paired PSUM.
        t = pools[tag].tile([120, 1028], BF16, tag=tag, name=tag)
        a1 = nc.scalar.activation(
            t[0:96, :].rearrange("p (h x) -> p h x", h=2)[:, :, 1:513],
            pt[:], RELU)
        z0 = nc.gpsimd.memset(
            t[0:96, :].rearrange("p (h x) -> p h x", h=2)[:, :, 0:514:513], 0.0)
        add_dep(z0, a1, reason="pad-cols")
        insts = [a1, z0]
        if s == -1 and 8 - k > 0:
            z = nc.sync.dma_start(t[0:(8 - k) * 12, :], zt[0:(8 - k) * 12, :])
            for a in (a1, z0):
                add_dep(z, a, reason="zero-pad-rows")
            insts.append(z)
        if s == 63 and 8 - k < 8:
            z = nc.sync.dma_start(t[(8 - k) * 12:96, :], zt[0:k * 12, :])
            for a in (a1, z0):
                add_dep(z, a, reason="zero-pad-rows")
            insts.append(z)
        store[s] = (t, insts)

    def halo(store, s, eng):
        # store[s][96:120] <- store[s+1][0:24]  (rows y+8, y+9)
        dst, insts = store[s]
        if s + 1 in store:
            d = eng.dma_start(dst[96:120, :], store[s + 1][0][0:24, :])
        else:
            d = eng.dma_start(dst[96:120, :], zt[0:24, :])
        for i_ in insts:
            add_dep(d, i_, reason="halo-after-evict")

    ph2_t = {}

    def phase2a(s, pt):
        # DVE/scalar part: i = relu(f3), E = exp(relu(w3)); the selector
        # matmuls run one iteration later (phase2b) so they never make the
        # PE queue wait on this cross-engine chain
        it = ph2.tile([96, 512], BF16, tag="i")
        et = ph2.tile([96, 512], BF16, tag="e")
        nc.vector.tensor_scalar_max(it[:], pt[:, 0:512], 0.0)
        nc.scalar.activation(et[:], pt[:, 512:1024], EXP)
        # exp(relu(x)) == max(exp(x), 1) -- this IS the last-layer relu
        # (gpsimd tensor ops are ~15x slower than DVE -- keep on vector)
        nc.vector.tensor_scalar_max(et[:], et[:], 1.0)
        nc.vector.tensor_mul(it[:], it[:], et[:])  # i*E in place
        ph2_t[s] = (it, et)

    def phase2b(s):
        it, et = ph2_t.pop(s)
        p2 = ps2.tile([24, 512], F32, tag="p2")
        nc.tensor.matmul(p2[:], selA_v, it[:], start=True, stop=False)
        nc.tensor.matmul(p2[:], selB_v, et[:], start=False, stop=True)
        s2 = ph2.tile([24, 512], F32, tag="s2")
        nc.vector.tensor_copy(s2[:], p2[:])
        ys = 8 * s + 3
        ya, yb = max(ys, 0), min(ys + 8, HW)
        while ya < yb:
            run = min(yb - ya, 128 - (ya % 128))
            p0 = ya % 128
            dst = g3[p0:p0 + run, :].rearrange(
                "p (s c x) -> p s c x", s=3, c=4)[:, :, ya // 128, :]
            sv = s2[(ya - ys) * 3:(ya - ys + run) * 3, :]
            nc.gpsimd.dma_start(dst, sv)
            ya += run

    # --- phases 2.5/3/4 as chunked functions, drip-fed into the wavefront
    asm_specs = [  # (ch, py, px, qp index or None, plane addend)
        (0, 0, 0, 0, 0), (0, 0, 1, None, 1),
        (0, 1, 0, 4, "g1"), (0, 1, 1, 3, 3),
        (1, 0, 0, None, 0), (1, 0, 1, None, "g0"),
        (1, 1, 0, None, "g1"), (1, 1, 1, None, 3),
        (2, 0, 0, 2, 0), (2, 0, 1, 5, "g0"),
        (2, 1, 0, None, 2), (2, 1, 1, 1, 3),
    ]
    qp6_h = {}

    def phase25(hc):
        # green + chroma for y rows 64*hc .. 64*hc+63
        t = hc // 2
        p0 = (hc % 2) * 64
        cs = slice(t * 512, (t + 1) * 512)
        pr = slice(p0, p0 + 64)
        rd = rden[pr, 0:512]
        nc.vector.reciprocal(rd, den[pr, cs])
        nc.vector.tensor_mul(g0p[pr, cs], g0n[pr, cs], rd)
        nc.vector.tensor_mul(g1p[pr, cs], g1n[pr, cs], rd)
        nc.vector.tensor_sub(c1p[pr, cs], mos_pl(1)[pr, cs], g0p[pr, cs])
        nc.vector.tensor_sub(c2p[pr, cs], mos_pl(2)[pr, cs], g1p[pr, cs])

    def phase3_strip(sq):
        h = sq // 16
        if h not in qp6_h:
            qp6_h[h] = qpp.tile([128, 6 * 1024], BF16, tag="qp6",
                                name=f"qp6_{h}")
        qp6 = qp6_h[h]
        b3 = pools["b3"].tile([64, 514], BF16, tag="b3", name="b3")
        wrs = [nc.gpsimd.memset(b3[0:64, 0:514:513], 0.0)]
        y0 = 16 * sq - 1
        if y0 < 0:
            wrs.append(nc.gpsimd.memset(b3[0:1, :], 0.0))
            wrs.append(nc.gpsimd.memset(b3[32:33, :], 0.0))
        if y0 + 18 > HW:
            wrs.append(nc.sync.dma_start(b3[17:18, :], zt[0:1, 0:514]))
            wrs.append(nc.sync.dma_start(b3[49:50, :], zt[0:1, 0:514]))
        for cc, src_plane in ((0, c1p), (1, c2p)):
            ya, yb = max(y0, 0), min(y0 + 18, HW)
            while ya < yb:
                run = min(yb - ya, 128 - (ya % 128))
                d = nc.sync.dma_start(
                    b3[cc * 32 + ya - y0:cc * 32 + ya - y0 + run, 1:513],
                    src_plane[ya % 128:ya % 128 + run,
                              (ya // 128) * 512:(ya // 128) * 512 + 512])
                wrs.append(d)
                ya += run
        wrs.append(nc.gpsimd.dma_start(b3[18:32, :], zt[0:14, 0:514]))
        wrs.append(nc.gpsimd.dma_start(b3[50:64, :], zt[0:14, 0:514]))
        p3 = ps3.tile([96, 512], F32, tag="p3", name="p3")
        mm3 = [nc.tensor.matmul(p3[:], w5_v(dx), b3[0:64, dx:dx + 512],
                                start=(dx == 0), stop=(dx == 2))
               for dx in range(3)]
        for mm in mm3:
            for wr in wrs:
                add_dep(mm, wr, reason="b3-ready")
        s3 = ph2.tile([96, 512], BF16, tag="s3")
        nc.scalar.copy(s3[:], p3[:])
        yq = 16 * sq
        tlc = (yq // 128) - 2 * h  # 0 or 1: 512-chunk within the half
        dst = qp6[yq % 128:yq % 128 + 16, :].rearrange(
            "p (o c x) -> p o c x", o=6, c=2)[:, :, tlc, :]
        nc.sync.dma_start(dst, s3[:])

    def asm_unit(t, ch, py):
        # one output plane [128 quad rows, 1024 cols] for plane-chunk t
        h, tl = t // 2, t % 2
        qp6 = qp6_h[h]
        a = asmp.tile([128, 1024], F32, tag="asm", name="asm")
        prev = None
        for (c_, py_, px, qo, addend) in asm_specs:
            if c_ != ch or py_ != py:
                continue
            if addend == "g0":
                ad = g0p[:, t * 512:(t + 1) * 512]
            elif addend == "g1":
                ad = g1p[:, t * 512:(t + 1) * 512]
            else:
                ad = mos_pl(addend)[:, t * 512:(t + 1) * 512]
            view = a[:].rearrange("p (x two) -> p two x", two=2)[:, px, :]
            if qo is None:
                w_ = nc.vector.tensor_copy(view, ad)
            else:
                w_ = nc.vector.tensor_add(
                    view,
                    qp6[:, qo * 1024 + tl * 512:qo * 1024 + tl * 512 + 512],
                    ad)
            if prev is not None:
                add_dep(w_, prev, reason="asm-interleave")
            prev = w_
        dst = out[ch].rearrange("(y two) x -> two y x", two=2)[
            py, t * 128:(t + 1) * 128, :]
        nc.sync.dma_start(dst, a[:])

    # ---------------- emission schedule
    # a FIFO of small background closures (phase25 pieces, phase3 strips,
    # assembly units) drip-fed into the wavefront: bounded per-iteration
    # engine load, so the PE never starves (HAM stays at full clock)
    import collections
    bg = collections.deque()  # entries: (ready_iteration, closure)
    done3 = -1
    emitted3 = set()
    cur_i = [0]

    def phase25_parts(hc):
        # reciprocal is split in 4 so no single DVE op blocks the queue
        t = hc // 2
        p0 = (hc % 2) * 64
        cs = slice(t * 512, (t + 1) * 512)
        pr = slice(p0, p0 + 64)
        rd = rden[pr, 0:512]

        def recip(j):
            def f():
                nc.vector.reciprocal(
                    rd[:, j * 128:(j + 1) * 128],
                    den[pr, t * 512 + j * 128:t * 512 + (j + 1) * 128])
            return f

        def part_b():
            nc.vector.tensor_mul(g0p[pr, cs], g0n[pr, cs], rd)
            nc.vector.tensor_sub(c1p[pr, cs], mos_pl(1)[pr, cs], g0p[pr, cs])

        def part_c():
            nc.vector.tensor_mul(g1p[pr, cs], g1n[pr, cs], rd)
            nc.vector.tensor_sub(c2p[pr, cs], mos_pl(2)[pr, cs], g1p[pr, cs])

        return [recip(0), recip(1), recip(2), recip(3), part_b, part_c]

    def note_phase3_done(sq):
        nonlocal done3
        emitted3.add(sq)
        while done3 + 1 in emitted3:
            done3 += 1
            if (done3 + 1) % 8 == 0:
                t = done3 // 8
                for ch in range(3):
                    for py in range(2):
                        bg.append((cur_i[0] + 1,
                                   lambda t=t, ch=ch, py=py: asm_unit(t, ch, py)))

    def unlock(hc):
        # after phase25(hc), chroma rows <= 64*hc+63 are valid; the +2 delay
        # keeps the gather DMAs from camping on the sync queue head while
        # phase25's DVE work is still draining
        nonlocal sq_next
        lim = 4 * hc + 2 if hc < 7 else 31
        while sq_next <= lim:
            def do3(sq=sq_next):
                phase3_strip(sq)
                note_phase3_done(sq)
            bg.append((cur_i[0] + 2, do3))
            sq_next += 1

    sq_next = 0
    load_b0(-1)

    for i in range(NSTRIP + 4):
        cur_i[0] = i
        s = i - 1  # L1 strip index
        if s + 1 <= 63:
            load_b0(s + 1)  # prefetch one iteration ahead
        if s <= 63:
            pt = conv_pair(b0_t[s][0], 40, lambda dx: w1_v(0, dx),
                           lambda dx: w1_v(1, dx), 0, deps=b0_t[s][1])
            evict_pair(pt, b1_t, s, "b1", 1)
            b0_t.pop(s - 1, None)
        # halo for b1[s-1] (consumed by L2 next iteration); for s == 64 the
        # source strip 64 doesn't exist -> zero fill via halo() fallback
        if s - 1 in b1_t:
            halo(b1_t, s - 1, nc.sync)
        t2 = s - 2  # L2 strip index
        if -1 <= t2 <= 63:
            bt = b1_t[t2][0]
            pt = conv_pair(bt, 120, lambda dx: w23_v(0, dx),
                           lambda dx: w23_v(1, dx), 514)
            evict_pair(pt, b2_t, t2, "b2", 2)
        if t2 - 1 in b2_t:
            halo(b2_t, t2 - 1, nc.gpsimd)
        t3 = s - 4  # L3 strip index
        if -1 <= t3 <= 63:
            bt = b2_t[t3][0]
            pt = conv_pair(bt, 120, lambda dx: w23_v(2, dx),
                           lambda dx: w23_v(3, dx), 514)
            phase2a(t3, pt)
            b1_t.pop(t3, None)
            b2_t.pop(t3 - 1, None)
        t3b = t3 - 1  # deferred selector matmuls + plane write
        if -1 <= t3b <= 63:
            phase2b(t3b)
            if t3b >= 7 and (t3b - 7) % 8 == 0:
                hc = (t3b - 7) // 8
                bg.extend((i, f) for f in phase25_parts(hc))
                unlock(hc)
        # drip background closures (FIFO; head must be past its ready-iter)
        for _ in range(3):
            if bg and bg[0][0] <= i:
                bg.popleft()[1]()

    phase2b(63)
    hc = 7
    bg.extend((0, f) for f in phase25_parts(hc))
    unlock(hc)
    # drain whatever is left (tail) -- no budget, engines pipeline freely
    while bg:
        bg.popleft()[1]()

    if "dbgp" in outs:
        dbgp = outs["dbgp"]  # [4, 512, 512] bf16
        for j, pl in enumerate((g0p, g1p, c1p, c2p)):
            nc.sync.dma_start(
                dbgp[j].rearrange("(t p) x -> p t x", p=128),
                pl.rearrange("p (t x) -> p t x", t=4))
        dbgq = outs["dbgq"]  # [2, 128, 6144] bf16
        for j, h in enumerate(sorted(qp6_h)):
            nc.sync.dma_start(dbgq[j], qp6_h[h][:])


_CACHE = {}


def _get_compiled():
    if "nc" in _CACHE:
        return _CACHE["nc"]
    nc = bacc.Bacc("TRN2", target_bir_lowering=False, debug=False,
                   enable_asserts=False)
    ins = {
        "mospad": nc.dram_tensor("mospad", [4, 522, 514], BF16,
                                 kind="ExternalInput").ap(),
        "wblob": nc.dram_tensor("wblob", [120, WBLOB_COLS], BF16,
                                kind="ExternalInput").ap(),
    }
    outs = {"out": nc.dram_tensor("out", [3, 1024, 1024], F32,
                                  kind="ExternalOutput").ap()}
    from contextlib import ExitStack
    with tile.TileContext(nc) as tc, ExitStack() as ctx:
        build_kernel(tc, outs, ins, ctx)
    nc.compile()
    _CACHE["nc"] = nc
    return nc


def kernel(**inputs):
    nc = _get_compiled()
    mospad, shared = _host_prep(inputs)
    in_maps = []
    for b in range(8):
        m = {"mospad": np.ascontiguousarray(mospad[b])}
        m.update(shared)
        in_maps.append(m)
    res = run_bass_kernel_spmd(nc, in_maps, core_ids=list(range(8)))
    return np.stack([res.results[b]["out"] for b in range(8)])


# revision 28
# speedup vs baseline: 2.0220x; 1.0328x over previous
"""Trainium2 Bass kernel for the BasicQuadRGBV2 demosaic model.

Data-parallel over batch: 1 image per NeuronCore (8 cores).

Per-core dataflow (image [4,512,512] -> [3,1024,1024]):
  Phase 1  (conv stacks): two 3-layer CNNs (4->12->12->12, 3x3, relu) computed
           as block-banded bf16 matmuls. Layout: partitions = (y_row_window x
           chan), free dim = x. The y-taps of each 3x3 conv live inside a
           banded lhsT (contract over (y_in, c)); the x-taps are 3
           PSUM-accumulated matmuls over free-dim-shifted views. Strips of 8
           output rows; the output grid drifts +1 row per layer so PSUM
           evictions always land at natural partitions; strip-to-strip halo
           rows move via small DMAs emitted one iteration ahead. The f and w
           stacks share one strip tile (f in cols 0:514, w in 514:1028) so
           each halo is a single DMA.
  Phase 2  (softmax green): E=exp(relu-free w3), i=relu(f3); selector matmuls
           reduce over channels-in-partitions giving g0num/g1num/den planes
           (f32).
  Phase 2.5 (per 64-row half-chunk): rden~=1/den (fast approx); g0,g1 (bf16);
           chroma c1=mosaic1-g0, c2=mosaic2-g1 (bf16).
  Phase 3  (chroma 5x5 convs): in pixel-shuffled space each needed
           (conv, phase) output is a 12-tap stencil over (c1,c2) within a
           3x3 quad-space window -> same banded-matmul machinery, 6 outputs
           at once.
  Phase 4  (assembly): DVE/scalar writes with stride-2 free APs interleave
           quad planes into full-res f32 rows; contiguous row DMAs to DRAM.

Phases 2.5/3/4 are drip-fed into the wavefront via ready-queues (bounded
work per iteration per engine) so the PE never idles long enough for the
HAM clock-gate to drop it to half rate.
"""

import numpy as np
import ml_dtypes

import concourse.bass as bass
import concourse.tile as tile
from concourse import bacc, mybir
from concourse.tile import add_dep_helper as _adh


def add_dep(frm, to, reason=""):
    _adh(frm.ins, to.ins, reason=reason)


from concourse.bass_utils import run_bass_kernel_spmd

F32 = mybir.dt.float32
BF16 = mybir.dt.bfloat16
RELU = mybir.ActivationFunctionType.Relu
EXP = mybir.ActivationFunctionType.Exp
COPY = mybir.ActivationFunctionType.Copy

WIDTH = 12
HW = 512  # image H = W (quad space)
NSTRIP = 65  # strips s = -1 .. 63, stride 8

# wblob column offsets
W1_OFS = 0            # [40, 6*96]   (st,dx)
W23_OFS = 576         # [120, 12*96] (ly,dx)
SEL_OFS = W23_OFS + 1152   # [96, 48]
W5_OFS = SEL_OFS + 48      # [64, 3*96]
WBLOB_COLS = W5_OFS + 288


# ---------------------------------------------------------------- host prep

def _band_lhsT(W, cin):
    """W: [12, cin, 3, 3] -> [3, 10*cin, 96] banded matrices (one per x-tap).

    lhsT_dx[(yi*cin + c), (yo*12 + oc)] = W[oc, c, yi - yo, dx]
    """
    K, M = 10 * cin, 8 * WIDTH
    out = np.zeros((3, K, M), np.float32)
    for dx in range(3):
        for yo in range(8):
            for dy in range(3):
                yi = yo + dy
                out[dx, yi * cin:(yi + 1) * cin, yo * WIDTH:(yo + 1) * WIDTH] = \
                    W[:, :, dy, dx].T
    return out


def _selectors():
    selA = np.zeros((96, 24), np.float32)  # applied to i*E
    selB = np.zeros((96, 24), np.float32)  # applied to E
    for yl in range(8):
        for c in range(WIDTH):
            p = yl * WIDTH + c
            if c < 6:
                selA[p, yl * 3 + 0] = 1.0
            else:
                selA[p, yl * 3 + 1] = 1.0
            selB[p, yl * 3 + 2] = 1.0
    return selA, selB


def _g_stencil(K5, py, px):
    """12-tap quad-space stencil of a 5x5 conv output at phase (py,px),
    over chroma channels c1 (phase (0,1)) and c2 (phase (1,0))."""
    G = np.zeros((2, 3, 3), np.float32)
    for cc, (qy, qx) in enumerate(((0, 1), (1, 0))):
        for dy in (-1, 0, 1):
            for dx in (-1, 0, 1):
                d5y = 2 * dy + 2 - py + qy
                d5x = 2 * dx + 2 - px + qx
                if 0 <= d5y < 5 and 0 <= d5x < 5:
                    G[cc, dy + 1, dx + 1] = K5[d5y, d5x]
    return G


def _chroma_lhsT(chw, cvw, cqw):
    """-> [3, 64, 96] banded matrices for the 6 (conv, phase) outputs.

    Output order o: 0 ch@(0,0), 1 ch@(1,1), 2 cv@(0,0), 3 cv@(1,1),
                    4 cq@(1,0), 5 cq@(0,1).
    """
    specs = [(chw, 0, 0), (chw, 1, 1), (cvw, 0, 0), (cvw, 1, 1),
             (cqw, 1, 0), (cqw, 0, 1)]
    out = np.zeros((3, 64, 96), np.float32)
    for o, (K5, py, px) in enumerate(specs):
        G = _g_stencil(np.asarray(K5)[0, 0], py, px)
        for dx in range(3):
            for yo in range(16):
                for dy in (-1, 0, 1):
                    yi = yo + dy + 1
                    for cc in range(2):
                        out[dx, cc * 32 + yi, yo * 6 + o] = G[cc, dy + 1, dx]
    return out


def _host_prep(inputs):
    mosaic = np.asarray(inputs["mosaic"], np.float32)  # [8,4,512,512]
    mospad = np.zeros((mosaic.shape[0], 4, 522, 514), ml_dtypes.bfloat16)
    mospad[:, :, 8:520, 1:513] = mosaic
    wblob = np.zeros((120, WBLOB_COLS), np.float32)
    w1 = [_band_lhsT(np.asarray(inputs["fw0"]), 4),
          _band_lhsT(np.asarray(inputs["ww0"]), 4)]
    for st in range(2):
        for dx in range(3):
            wblob[0:40, W1_OFS + (st * 3 + dx) * 96:
                  W1_OFS + (st * 3 + dx + 1) * 96] = w1[st][dx]
    w23 = [_band_lhsT(np.asarray(inputs["fw1"]), 12),
           _band_lhsT(np.asarray(inputs["ww1"]), 12),
           _band_lhsT(np.asarray(inputs["fw2"]), 12),
           _band_lhsT(np.asarray(inputs["ww2"]), 12)]
    for ly in range(4):
        for dx in range(3):
            wblob[0:120, W23_OFS + (ly * 3 + dx) * 96:
                  W23_OFS + (ly * 3 + dx + 1) * 96] = w23[ly][dx]
    selA, selB = _selectors()
    wblob[0:96, SEL_OFS:SEL_OFS + 24] = selA
    wblob[0:96, SEL_OFS + 24:SEL_OFS + 48] = selB
    w5 = _chroma_lhsT(inputs["chw"], inputs["cvw"], inputs["cqw"])
    for dx in range(3):
        wblob[0:64, W5_OFS + dx * 96:W5_OFS + (dx + 1) * 96] = w5[dx]
    return mospad, {"wblob": wblob.astype(ml_dtypes.bfloat16)}


# ---------------------------------------------------------------- kernel IR

def build_kernel(tc, outs, ins, ctx):
    nc = tc.nc
    mospad, wblob = ins["mospad"], ins["wblob"]
    out = outs["out"]

    wp = ctx.enter_context(tc.tile_pool(name="weights", bufs=1))
    pp = ctx.enter_context(tc.tile_pool(name="planes", bufs=1))
    # conv layer pairs accumulate f|w into one 2-bank PSUM tile -> single
    # paired eviction ACTIVATE; 3 pairs (L1,L2,L3) + p2 + p3 = 8 banks
    psp = ctx.enter_context(tc.tile_pool(name="psp", bufs=3, space="PSUM"))
    ps2 = ctx.enter_context(tc.tile_pool(name="ps2", bufs=1, space="PSUM"))
    ps3 = ctx.enter_context(tc.tile_pool(name="ps3", bufs=1, space="PSUM"))
    pools = {}
    for tag, b in (("b0", 4), ("b1", 4), ("b2", 4), ("b3", 6)):
        pools[tag] = ctx.enter_context(tc.tile_pool(name=f"p_{tag}", bufs=b))
    ph2 = ctx.enter_context(tc.tile_pool(name="ph2", bufs=3))
    qpp = ctx.enter_context(tc.tile_pool(name="qp", bufs=2))
    asmp = ctx.enter_context(tc.tile_pool(name="asm", bufs=4))

    # --- weights to SBUF (single DMA)
    wb = wp.tile([120, WBLOB_COLS], BF16, tag="wb")
    nc.sync.dma_start(wb[:], wblob)

    def w1_v(st, dx):
        return wb[0:40, W1_OFS + (st * 3 + dx) * 96:W1_OFS + (st * 3 + dx + 1) * 96]

    def w23_v(ly, dx):
        return wb[0:120, W23_OFS + (ly * 3 + dx) * 96:
                  W23_OFS + (ly * 3 + dx + 1) * 96]

    selA_v = wb[0:96, SEL_OFS:SEL_OFS + 24]
    selB_v = wb[0:96, SEL_OFS + 24:SEL_OFS + 48]

    def w5_v(dx):
        return wb[0:64, W5_OFS + dx * 96:W5_OFS + (dx + 1) * 96]

    # --- persistent planes [128, 2048]: y -> (y%128, (y//128)*512 + x)
    mosp = pp.tile([128, 4 * 2048], BF16, tag="mosp")
    for c in range(4):
        nc.sync.dma_start(
            mosp[:, c * 2048:(c + 1) * 2048].rearrange(
                "p (t x) -> p t x", t=4),
            mospad[c, 8:520, 1:513].rearrange("(t p) x -> p t x", p=128))

    def mos_pl(c):
        return mosp[:, c * 2048:(c + 1) * 2048]

    g3 = pp.tile([128, 3 * 2048], F32, tag="g3")
    g0n = g3[:, 0:2048]
    g1n = g3[:, 2048:4096]
    den = g3[:, 4096:6144]
    g01 = pp.tile([128, 2 * 2048], BF16, tag="g01")
    g0p = g01[:, 0:2048]
    g1p = g01[:, 2048:4096]
    c12 = pp.tile([128, 2 * 2048], BF16, tag="c12")
    c1p = c12[:, 0:2048]
    c2p = c12[:, 2048:4096]
    rden = pp.tile([128, 512], F32, tag="rden")
    zt = pp.tile([96, 1028], BF16, tag="zt")
    nc.gpsimd.memset(zt[:], 0.0)

    # --- phase 1+2 wavefront over strips
    b0_t, b1_t, b2_t = {}, {}, {}

    def load_b0(s):
        t = pools["b0"].tile([40, 514], BF16, tag="b0", name="b0")
        src = mospad[:, 8 * s + 8:8 * s + 18, :].transpose([1, 0, 2])
        d = nc.sync.dma_start(t[:], src)
        b0_t[s] = (t, [d])

    def conv_pair(rhs_tile, kdim, wvf, wvw, cofs_w, deps=()):
        # f stack -> psum cols 0:512, w stack -> 512:1024 (adjacent banks)
        pt = psp.tile([96, 1024], F32, tag="cp2", name="cp2")
        mms = []
        for half, (wv, cofs) in enumerate(((wvf, 0), (wvw, cofs_w))):
            for dx in range(3):
                mms.append(nc.tensor.matmul(
                    pt[:, half * 512:half * 512 + 512], wv(dx),
                    rhs_tile[0:kdim, cofs + dx:cofs + dx + 512],
                    start=(dx == 0), stop=(dx == 2)))
        for mm in mms:
            for dep in deps:
                add_dep(mm, dep, reason="rhs-ready")
        return pt

    def evict_pair(pt, store, s, tag, k):
        # strip rows m=0..7 hold y = 8s+k+m; rows outside [0,512) must be
        # exactly zero (conv zero-padding) or they leak into the next layer.
        # f stack -> cols 1:513, w stack -> 515:1027 of one tile, written by
        # a single two-segment ACTIVATE from the paired PSUM.
        t = pools[tag].tile([120, 1028], BF16, tag=tag, name=tag)
        a1 = nc.scalar.activation(
            t[0:96, :].rearrange("p (h x) -> p h x", h=2)[:, :, 1:513],
            pt[:], RELU)
        z0 = nc.gpsimd.memset(
            t[0:96, :].rearrange("p (h x) -> p h x", h=2)[:, :, 0:514:513], 0.0)
        add_dep(z0, a1, reason="pad-cols")
        insts = [a1, z0]
        if s == -1 and 8 - k > 0:
            z = nc.sync.dma_start(t[0:(8 - k) * 12, :], zt[0:(8 - k) * 12, :])
            for a in (a1, z0):
                add_dep(z, a, reason="zero-pad-rows")
            insts.append(z)
        if s == 63 and 8 - k < 8:
            z = nc.sync.dma_start(t[(8 - k) * 12:96, :], zt[0:k * 12, :])
            for a in (a1, z0):
                add_dep(z, a, reason="zero-pad-rows")
            insts.append(z)
        store[s] = (t, insts)

    def halo(store, s, eng):
        # store[s][96:120] <- store[s+1][0:24]  (rows y+8, y+9)
        dst, insts = store[s]
        if s + 1 in store:
            d = eng.dma_start(dst[96:120, :], store[s + 1][0][0:24, :])
        else:
            d = eng.dma_start(dst[96:120, :], zt[0:24, :])
        for i_ in insts:
            add_dep(d, i_, reason="halo-after-evict")

    ph2_t = {}

    def phase2a(s, pt):
        # DVE/scalar part: i = relu(f3), E = exp(relu(w3)); the selector
        # matmuls run one iteration later (phase2b) so they never make the
        # PE queue wait on this cross-engine chain
        it = ph2.tile([96, 512], BF16, tag="i")
        et = ph2.tile([96, 512], BF16, tag="e")
        nc.vector.tensor_scalar_max(it[:], pt[:, 0:512], 0.0)
        nc.scalar.activation(et[:], pt[:, 512:1024], EXP)
        # exp(relu(x)) == max(exp(x), 1) -- this IS the last-layer relu
        # (gpsimd tensor ops are ~15x slower than DVE -- keep on vector)
        nc.vector.tensor_scalar_max(et[:], et[:], 1.0)
        nc.vector.tensor_mul(it[:], it[:], et[:])  # i*E in place
        ph2_t[s] = (it, et)

    def phase2b(s):
        it, et = ph2_t.pop(s)
        p2 = ps2.tile([24, 512], F32, tag="p2")
        nc.tensor.matmul(p2[:], selA_v, it[:], start=True, stop=False)
        nc.tensor.matmul(p2[:], selB_v, et[:], start=False, stop=True)
        s2 = ph2.tile([24, 512], F32, tag="s2")
        nc.vector.tensor_copy(s2[:], p2[:])
        ys = 8 * s + 3
        ya, yb = max(ys, 0), min(ys + 8, HW)
        while ya < yb:
            run = min(yb - ya, 128 - (ya % 128))
            p0 = ya % 128
            dst = g3[p0:p0 + run, :].rearrange(
                "p (s c x) -> p s c x", s=3, c=4)[:, :, ya // 128, :]
            sv = s2[(ya - ys) * 3:(ya - ys + run) * 3, :]
            nc.gpsimd.dma_start(dst, sv)
            ya += run

    # --- phases 2.5/3/4 as chunked functions, drip-fed into the wavefront
    asm_specs = [  # (ch, py, px, qp index or None, plane addend)
        (0, 0, 0, 0, 0), (0, 0, 1, None, 1),
        (0, 1, 0, 4, "g1"), (0, 1, 1, 3, 3),
        (1, 0, 0, None, 0), (1, 0, 1, None, "g0"),
        (1, 1, 0, None, "g1"), (1, 1, 1, None, 3),
        (2, 0, 0, 2, 0), (2, 0, 1, 5, "g0"),
        (2, 1, 0, None, 2), (2, 1, 1, 1, 3),
    ]
    qp6_h = {}

    def phase25(hc):
        # green + chroma for y rows 64*hc .. 64*hc+63
        t = hc // 2
        p0 = (hc % 2) * 64
        cs = slice(t * 512, (t + 1) * 512)
        pr = slice(p0, p0 + 64)
        rd = rden[pr, 0:512]
        nc.vector.reciprocal(rd, den[pr, cs])
        nc.vector.tensor_mul(g0p[pr, cs], g0n[pr, cs], rd)
        nc.vector.tensor_mul(g1p[pr, cs], g1n[pr, cs], rd)
        nc.vector.tensor_sub(c1p[pr, cs], mos_pl(1)[pr, cs], g0p[pr, cs])
        nc.vector.tensor_sub(c2p[pr, cs], mos_pl(2)[pr, cs], g1p[pr, cs])

    p3_state = {}

    def phase3_prep(sq):
        # gather the 18-row (c1,c2) band into b3 well before its matmuls
        b3 = pools["b3"].tile([64, 514], BF16, tag="b3", name="b3")
        wrs = [nc.gpsimd.memset(b3[0:64, 0:514:513], 0.0)]
        y0 = 16 * sq - 1
        if y0 < 0:
            wrs.append(nc.gpsimd.memset(b3[0:1, :], 0.0))
            wrs.append(nc.gpsimd.memset(b3[32:33, :], 0.0))
        if y0 + 18 > HW:
            wrs.append(nc.sync.dma_start(b3[17:18, :], zt[0:1, 0:514]))
            wrs.append(nc.sync.dma_start(b3[49:50, :], zt[0:1, 0:514]))
        for cc, src_plane in ((0, c1p), (1, c2p)):
            ya, yb = max(y0, 0), min(y0 + 18, HW)
            while ya < yb:
                run = min(yb - ya, 128 - (ya % 128))
                d = nc.sync.dma_start(
                    b3[cc * 32 + ya - y0:cc * 32 + ya - y0 + run, 1:513],
                    src_plane[ya % 128:ya % 128 + run,
                              (ya // 128) * 512:(ya // 128) * 512 + 512])
                wrs.append(d)
                ya += run
        wrs.append(nc.gpsimd.dma_start(b3[18:32, :], zt[0:14, 0:514]))
        wrs.append(nc.gpsimd.dma_start(b3[50:64, :], zt[0:14, 0:514]))
        p3_state[sq] = (b3, wrs)

    def phase3_strip(sq):
        h = sq // 16
        if h not in qp6_h:
            qp6_h[h] = qpp.tile([128, 6 * 1024], BF16, tag="qp6",
                                name=f"qp6_{h}")
        qp6 = qp6_h[h]
        b3, wrs = p3_state.pop(sq)
        p3 = ps3.tile([96, 512], F32, tag="p3", name="p3")
        mm3 = [nc.tensor.matmul(p3[:], w5_v(dx), b3[0:64, dx:dx + 512],
                                start=(dx == 0), stop=(dx == 2))
               for dx in range(3)]
        for mm in mm3:
            for wr in wrs:
                add_dep(mm, wr, reason="b3-ready")
        s3 = ph2.tile([96, 512], BF16, tag="s3")
        nc.scalar.copy(s3[:], p3[:])
        yq = 16 * sq
        tlc = (yq // 128) - 2 * h  # 0 or 1: 512-chunk within the half
        dst = qp6[yq % 128:yq % 128 + 16, :].rearrange(
            "p (o c x) -> p o c x", o=6, c=2)[:, :, tlc, :]
        nc.sync.dma_start(dst, s3[:])

    def asm_unit(t, ch, py):
        # one output plane [128 quad rows, 1024 cols] for plane-chunk t
        h, tl = t // 2, t % 2
        qp6 = qp6_h[h]
        a = asmp.tile([128, 1024], F32, tag="asm", name="asm")
        prev = None
        for (c_, py_, px, qo, addend) in asm_specs:
            if c_ != ch or py_ != py:
                continue
            if addend == "g0":
                ad = g0p[:, t * 512:(t + 1) * 512]
            elif addend == "g1":
                ad = g1p[:, t * 512:(t + 1) * 512]
            else:
                ad = mos_pl(addend)[:, t * 512:(t + 1) * 512]
            view = a[:].rearrange("p (x two) -> p two x", two=2)[:, px, :]
            if qo is None:
                w_ = nc.vector.tensor_copy(view, ad)
            else:
                w_ = nc.vector.tensor_add(
                    view,
                    qp6[:, qo * 1024 + tl * 512:qo * 1024 + tl * 512 + 512],
                    ad)
            if prev is not None:
                add_dep(w_, prev, reason="asm-interleave")
            prev = w_
        dst = out[ch].rearrange("(y two) x -> two y x", two=2)[
            py, t * 128:(t + 1) * 128, :]
        nc.sync.dma_start(dst, a[:])

    # ---------------- emission schedule
    # a FIFO of small background closures (phase25 pieces, phase3 strips,
    # assembly units) drip-fed into the wavefront: bounded per-iteration
    # engine load, so the PE never starves (HAM stays at full clock)
    import collections
    bg = collections.deque()  # entries: (ready_iteration, closure)
    done3 = -1
    emitted3 = set()
    cur_i = [0]

    def phase25_parts(hc):
        # reciprocal is split in 4 so no single DVE op blocks the queue
        t = hc // 2
        p0 = (hc % 2) * 64
        cs = slice(t * 512, (t + 1) * 512)
        pr = slice(p0, p0 + 64)
        rd = rden[pr, 0:512]

        def recip(j):
            def f():
                nc.vector.reciprocal(
                    rd[:, j * 128:(j + 1) * 128],
                    den[pr, t * 512 + j * 128:t * 512 + (j + 1) * 128])
            return f

        def part_b():
            nc.vector.tensor_mul(g0p[pr, cs], g0n[pr, cs], rd)
            nc.vector.tensor_sub(c1p[pr, cs], mos_pl(1)[pr, cs], g0p[pr, cs])

        def part_c():
            nc.vector.tensor_mul(g1p[pr, cs], g1n[pr, cs], rd)
            nc.vector.tensor_sub(c2p[pr, cs], mos_pl(2)[pr, cs], g1p[pr, cs])

        return [recip(0), recip(1), recip(2), recip(3), part_b, part_c]

    def note_phase3_done(sq):
        nonlocal done3
        emitted3.add(sq)
        while done3 + 1 in emitted3:
            done3 += 1
            if (done3 + 1) % 8 == 0:
                t = done3 // 8
                for ch in range(3):
                    for py in range(2):
                        bg.append((cur_i[0] + 1,
                                   lambda t=t, ch=ch, py=py: asm_unit(t, ch, py)))

    def unlock(hc):
        # after phase25(hc), chroma rows <= 64*hc+63 are valid; preps go two
        # iterations out (past the phase25 DVE drain), their matmuls one
        # more -- so phase3 MMs never wait at the head of the PE queue
        nonlocal sq_next
        lim = 4 * hc + 2 if hc < 7 else 31
        batch = list(range(sq_next, lim + 1))
        sq_next = lim + 1
        base = cur_i[0]
        for sq in batch:
            bg.append((base + 2, lambda sq=sq: phase3_prep(sq)))
        for sq in batch:
            def do3(sq=sq):
                phase3_strip(sq)
                note_phase3_done(sq)
            bg.append((base + 3, do3))

    sq_next = 0
    load_b0(-1)

    for i in range(NSTRIP + 4):
        cur_i[0] = i
        s = i - 1  # L1 strip index
        if s + 1 <= 63:
            load_b0(s + 1)  # prefetch one iteration ahead
        if s <= 63:
            pt = conv_pair(b0_t[s][0], 40, lambda dx: w1_v(0, dx),
                           lambda dx: w1_v(1, dx), 0, deps=b0_t[s][1])
            evict_pair(pt, b1_t, s, "b1", 1)
            b0_t.pop(s - 1, None)
        # halo for b1[s-1] (consumed by L2 next iteration); for s == 64 the
        # source strip 64 doesn't exist -> zero fill via halo() fallback
        if s - 1 in b1_t:
            halo(b1_t, s - 1, nc.sync)
        t2 = s - 2  # L2 strip index
        if -1 <= t2 <= 63:
            bt = b1_t[t2][0]
            pt = conv_pair(bt, 120, lambda dx: w23_v(0, dx),
                           lambda dx: w23_v(1, dx), 514)
            evict_pair(pt, b2_t, t2, "b2", 2)
        if t2 - 1 in b2_t:
            halo(b2_t, t2 - 1, nc.gpsimd)
        t3 = s - 4  # L3 strip index
        if -1 <= t3 <= 63:
            bt = b2_t[t3][0]
            pt = conv_pair(bt, 120, lambda dx: w23_v(2, dx),
                           lambda dx: w23_v(3, dx), 514)
            phase2a(t3, pt)
            b1_t.pop(t3, None)
            b2_t.pop(t3 - 1, None)
        t3b = t3 - 1  # deferred selector matmuls + plane write
        if -1 <= t3b <= 63:
            phase2b(t3b)
            if t3b >= 7 and (t3b - 7) % 8 == 0:
                hc = (t3b - 7) // 8
                bg.extend((i, f) for f in phase25_parts(hc))
                unlock(hc)
        # drip background closures (FIFO; head must be past its ready-iter)
        for _ in range(3):
            if bg and bg[0][0] <= i:
                bg.popleft()[1]()

    phase2b(63)
    hc = 7
    bg.extend((0, f) for f in phase25_parts(hc))
    unlock(hc)
    # drain whatever is left (tail) -- no budget, engines pipeline freely
    while bg:
        bg.popleft()[1]()

    if "dbgp" in outs:
        dbgp = outs["dbgp"]  # [4, 512, 512] bf16
        for j, pl in enumerate((g0p, g1p, c1p, c2p)):
            nc.sync.dma_start(
                dbgp[j].rearrange("(t p) x -> p t x", p=128),
                pl.rearrange("p (t x) -> p t x", t=4))
        dbgq = outs["dbgq"]  # [2, 128, 6144] bf16
        for j, h in enumerate(sorted(qp6_h)):
            nc.sync.dma_start(dbgq[j], qp6_h[h][:])


_CACHE = {}


def _get_compiled():
    if "nc" in _CACHE:
        return _CACHE["nc"]
    nc = bacc.Bacc("TRN2", target_bir_lowering=False, debug=False,
                   enable_asserts=False)
    ins = {
        "mospad": nc.dram_tensor("mospad", [4, 522, 514], BF16,
                                 kind="ExternalInput").ap(),
        "wblob": nc.dram_tensor("wblob", [120, WBLOB_COLS], BF16,
                                kind="ExternalInput").ap(),
    }
    outs = {"out": nc.dram_tensor("out", [3, 1024, 1024], F32,
                                  kind="ExternalOutput").ap()}
    from contextlib import ExitStack
    with tile.TileContext(nc) as tc, ExitStack() as ctx:
        build_kernel(tc, outs, ins, ctx)
    nc.compile()
    _CACHE["nc"] = nc
    return nc


def kernel(**inputs):
    nc = _get_compiled()
    mospad, shared = _host_prep(inputs)
    in_maps = []
    for b in range(8):
        m = {"mospad": np.ascontiguousarray(mospad[b])}
        m.update(shared)
        in_maps.append(m)
    res = run_bass_kernel_spmd(nc, in_maps, core_ids=list(range(8)))
    return np.stack([res.results[b]["out"] for b in range(8)])


# revision 33
# speedup vs baseline: 2.1711x; 1.0737x over previous
"""Trainium2 Bass kernel for the BasicQuadRGBV2 demosaic model.

Data-parallel over batch: 1 image per NeuronCore (8 cores).

Per-core dataflow (image [4,512,512] -> [3,1024,1024]):
  Phase 1  (conv stacks): two 3-layer CNNs (4->12->12->12, 3x3, relu) computed
           as block-banded bf16 matmuls. Layout: partitions = (y_row_window x
           chan), free dim = x. The y-taps of each 3x3 conv live inside a
           banded lhsT (contract over (y_in, c)); the x-taps are 3
           PSUM-accumulated matmuls over free-dim-shifted views. Strips of 8
           output rows; the output grid drifts +1 row per layer so PSUM
           evictions always land at natural partitions; strip-to-strip halo
           rows move via small DMAs emitted one iteration ahead. The f and w
           stacks share one strip tile (f in cols 0:514, w in 514:1028) so
           each halo is a single DMA.
  Phase 2  (softmax green): E=exp(relu-free w3), i=relu(f3); selector matmuls
           reduce over channels-in-partitions giving g0num/g1num/den planes
           (f32).
  Phase 2.5 (per 64-row half-chunk): rden~=1/den (fast approx); g0,g1 (bf16);
           chroma c1=mosaic1-g0, c2=mosaic2-g1 (bf16).
  Phase 3  (chroma 5x5 convs): in pixel-shuffled space each needed
           (conv, phase) output is a 12-tap stencil over (c1,c2) within a
           3x3 quad-space window -> same banded-matmul machinery, 6 outputs
           at once.
  Phase 4  (assembly): DVE/scalar writes with stride-2 free APs interleave
           quad planes into full-res f32 rows; contiguous row DMAs to DRAM.

Phases 2.5/3/4 are drip-fed into the wavefront via ready-queues (bounded
work per iteration per engine) so the PE never idles long enough for the
HAM clock-gate to drop it to half rate.
"""

import numpy as np
import ml_dtypes

import concourse.bass as bass
import concourse.tile as tile
from concourse import bacc, mybir
from concourse.tile import add_dep_helper as _adh


def add_dep(frm, to, reason=""):
    _adh(frm.ins, to.ins, reason=reason)


from concourse.bass_utils import run_bass_kernel_spmd

F32 = mybir.dt.float32
BF16 = mybir.dt.bfloat16
RELU = mybir.ActivationFunctionType.Relu
EXP = mybir.ActivationFunctionType.Exp
COPY = mybir.ActivationFunctionType.Copy

WIDTH = 12
HW = 512  # image H = W (quad space)
NSTRIP = 65  # strips s = -1 .. 63, stride 8

# wblob column offsets
W1_OFS = 0            # [40, 6*96]   (st,dx)
W23_OFS = 576         # [120, 12*96] (ly,dx)
SEL_OFS = W23_OFS + 1152   # [96, 48]
W5_OFS = SEL_OFS + 48      # [64, 3*96]
WBLOB_COLS = W5_OFS + 288


# ---------------------------------------------------------------- host prep

def _band_lhsT(W, cin):
    """W: [12, cin, 3, 3] -> [3, 10*cin, 96] banded matrices (one per x-tap).

    lhsT_dx[(yi*cin + c), (yo*12 + oc)] = W[oc, c, yi - yo, dx]
    """
    K, M = 10 * cin, 8 * WIDTH
    out = np.zeros((3, K, M), np.float32)
    for dx in range(3):
        for yo in range(8):
            for dy in range(3):
                yi = yo + dy
                out[dx, yi * cin:(yi + 1) * cin, yo * WIDTH:(yo + 1) * WIDTH] = \
                    W[:, :, dy, dx].T
    return out


def _selectors():
    selA = np.zeros((96, 24), np.float32)  # applied to i*E
    selB = np.zeros((96, 24), np.float32)  # applied to E
    for yl in range(8):
        for c in range(WIDTH):
            p = yl * WIDTH + c
            if c < 6:
                selA[p, yl * 3 + 0] = 1.0
            else:
                selA[p, yl * 3 + 1] = 1.0
            selB[p, yl * 3 + 2] = 1.0
    return selA, selB


def _g_stencil(K5, py, px):
    """12-tap quad-space stencil of a 5x5 conv output at phase (py,px),
    over chroma channels c1 (phase (0,1)) and c2 (phase (1,0))."""
    G = np.zeros((2, 3, 3), np.float32)
    for cc, (qy, qx) in enumerate(((0, 1), (1, 0))):
        for dy in (-1, 0, 1):
            for dx in (-1, 0, 1):
                d5y = 2 * dy + 2 - py + qy
                d5x = 2 * dx + 2 - px + qx
                if 0 <= d5y < 5 and 0 <= d5x < 5:
                    G[cc, dy + 1, dx + 1] = K5[d5y, d5x]
    return G


def _chroma_lhsT(chw, cvw, cqw):
    """-> [3, 64, 96] banded matrices for the 6 (conv, phase) outputs.

    Output order o: 0 ch@(0,0), 1 ch@(1,1), 2 cv@(0,0), 3 cv@(1,1),
                    4 cq@(1,0), 5 cq@(0,1).
    """
    specs = [(chw, 0, 0), (chw, 1, 1), (cvw, 0, 0), (cvw, 1, 1),
             (cqw, 1, 0), (cqw, 0, 1)]
    out = np.zeros((3, 64, 96), np.float32)
    for o, (K5, py, px) in enumerate(specs):
        G = _g_stencil(np.asarray(K5)[0, 0], py, px)
        for dx in range(3):
            for yo in range(16):
                for dy in (-1, 0, 1):
                    yi = yo + dy + 1
                    for cc in range(2):
                        out[dx, cc * 32 + yi, yo * 6 + o] = G[cc, dy + 1, dx]
    return out


def _host_prep(inputs):
    mosaic = np.asarray(inputs["mosaic"], np.float32)  # [8,4,512,512]
    mospad = np.zeros((mosaic.shape[0], 4, 522, 514), ml_dtypes.bfloat16)
    mospad[:, :, 8:520, 1:513] = mosaic
    wblob = np.zeros((120, WBLOB_COLS), np.float32)
    w1 = [_band_lhsT(np.asarray(inputs["fw0"]), 4),
          _band_lhsT(np.asarray(inputs["ww0"]), 4)]
    for st in range(2):
        for dx in range(3):
            wblob[0:40, W1_OFS + (st * 3 + dx) * 96:
                  W1_OFS + (st * 3 + dx + 1) * 96] = w1[st][dx]
    w23 = [_band_lhsT(np.asarray(inputs["fw1"]), 12),
           _band_lhsT(np.asarray(inputs["ww1"]), 12),
           _band_lhsT(np.asarray(inputs["fw2"]), 12),
           _band_lhsT(np.asarray(inputs["ww2"]), 12)]
    for ly in range(4):
        for dx in range(3):
            wblob[0:120, W23_OFS + (ly * 3 + dx) * 96:
                  W23_OFS + (ly * 3 + dx + 1) * 96] = w23[ly][dx]
    selA, selB = _selectors()
    wblob[0:96, SEL_OFS:SEL_OFS + 24] = selA
    wblob[0:96, SEL_OFS + 24:SEL_OFS + 48] = selB
    w5 = _chroma_lhsT(inputs["chw"], inputs["cvw"], inputs["cqw"])
    for dx in range(3):
        wblob[0:64, W5_OFS + dx * 96:W5_OFS + (dx + 1) * 96] = w5[dx]
    return mospad, {"wblob": wblob.astype(ml_dtypes.bfloat16)}


# ---------------------------------------------------------------- kernel IR

def build_kernel(tc, outs, ins, ctx):
    nc = tc.nc
    mospad, wblob = ins["mospad"], ins["wblob"]
    out = outs["out"]

    wp = ctx.enter_context(tc.tile_pool(name="weights", bufs=1))
    pp = ctx.enter_context(tc.tile_pool(name="planes", bufs=1))
    # conv layer pairs accumulate f|w into one 2-bank PSUM tile -> single
    # paired eviction ACTIVATE; 3 pairs (L1,L2,L3) + p2 + p3 = 8 banks
    psp = ctx.enter_context(tc.tile_pool(name="psp", bufs=3, space="PSUM"))
    ps2 = ctx.enter_context(tc.tile_pool(name="ps2", bufs=1, space="PSUM"))
    ps3 = ctx.enter_context(tc.tile_pool(name="ps3", bufs=1, space="PSUM"))
    pools = {}
    for tag, b in (("b0", 4), ("b1", 4), ("b2", 4), ("b3", 6)):
        pools[tag] = ctx.enter_context(tc.tile_pool(name=f"p_{tag}", bufs=b))
    ph2 = ctx.enter_context(tc.tile_pool(name="ph2", bufs=3))
    qpp = ctx.enter_context(tc.tile_pool(name="qp", bufs=2))
    asmp = ctx.enter_context(tc.tile_pool(name="asm", bufs=4))

    # --- weights to SBUF (single DMA)
    wb = wp.tile([120, WBLOB_COLS], BF16, tag="wb")
    nc.sync.dma_start(wb[:], wblob)

    def w1_v(st, dx):
        return wb[0:40, W1_OFS + (st * 3 + dx) * 96:W1_OFS + (st * 3 + dx + 1) * 96]

    def w23_v(ly, dx):
        return wb[0:120, W23_OFS + (ly * 3 + dx) * 96:
                  W23_OFS + (ly * 3 + dx + 1) * 96]

    selA_v = wb[0:96, SEL_OFS:SEL_OFS + 24]
    selB_v = wb[0:96, SEL_OFS + 24:SEL_OFS + 48]

    def w5_v(dx):
        return wb[0:64, W5_OFS + dx * 96:W5_OFS + (dx + 1) * 96]

    # --- persistent planes [128, 2048]: y -> (y%128, (y//128)*512 + x)
    mosp = pp.tile([128, 4 * 2048], BF16, tag="mosp")
    for c in range(4):
        nc.sync.dma_start(
            mosp[:, c * 2048:(c + 1) * 2048].rearrange(
                "p (t x) -> p t x", t=4),
            mospad[c, 8:520, 1:513].rearrange("(t p) x -> p t x", p=128))

    def mos_pl(c):
        return mosp[:, c * 2048:(c + 1) * 2048]

    g3 = pp.tile([128, 3 * 2048], F32, tag="g3")
    g0n = g3[:, 0:2048]
    g1n = g3[:, 2048:4096]
    den = g3[:, 4096:6144]
    g01 = pp.tile([128, 2 * 2048], BF16, tag="g01")
    g0p = g01[:, 0:2048]
    g1p = g01[:, 2048:4096]
    c12 = pp.tile([128, 2 * 2048], BF16, tag="c12")
    c1p = c12[:, 0:2048]
    c2p = c12[:, 2048:4096]
    rden = pp.tile([128, 512], F32, tag="rden")
    zt = pp.tile([96, 1028], BF16, tag="zt")
    nc.gpsimd.memset(zt[:], 0.0)

    # --- phase 1+2 wavefront over strips
    b0_t, b1_t, b2_t = {}, {}, {}

    def load_b0(s):
        t = pools["b0"].tile([40, 514], BF16, tag="b0", name="b0")
        src = mospad[:, 8 * s + 8:8 * s + 18, :].transpose([1, 0, 2])
        d = nc.sync.dma_start(t[:], src)
        b0_t[s] = (t, [d])

    def conv_pair(rhs_tile, kdim, wvf, wvw, cofs_w, deps=()):
        # f stack -> psum cols 0:512, w stack -> 512:1024 (adjacent banks)
        pt = psp.tile([96, 1024], F32, tag="cp2", name="cp2")
        mms = []
        for half, (wv, cofs) in enumerate(((wvf, 0), (wvw, cofs_w))):
            for dx in range(3):
                mms.append(nc.tensor.matmul(
                    pt[:, half * 512:half * 512 + 512], wv(dx),
                    rhs_tile[0:kdim, cofs + dx:cofs + dx + 512],
                    start=(dx == 0), stop=(dx == 2)))
        for mm in mms:
            for dep in deps:
                add_dep(mm, dep, reason="rhs-ready")
        return pt

    def evict_pair(pt, store, s, tag, k):
        # strip rows m=0..7 hold y = 8s+k+m; rows outside [0,512) must be
        # exactly zero (conv zero-padding) or they leak into the next layer.
        # f stack -> cols 1:513, w stack -> 515:1027 of one tile, written by
        # a single two-segment ACTIVATE from the paired PSUM.
        t = pools[tag].tile([120, 1028], BF16, tag=tag, name=tag)
        a1 = nc.scalar.activation(
            t[0:96, :].rearrange("p (h x) -> p h x", h=2)[:, :, 1:513],
            pt[:], RELU)
        z0 = nc.gpsimd.memset(
            t[0:96, :].rearrange("p (h x) -> p h x", h=2)[:, :, 0:514:513], 0.0)
        add_dep(z0, a1, reason="pad-cols")
        insts = [a1, z0]
        if s == -1 and 8 - k > 0:
            z = nc.sync.dma_start(t[0:(8 - k) * 12, :], zt[0:(8 - k) * 12, :])
            for a in (a1, z0):
                add_dep(z, a, reason="zero-pad-rows")
            insts.append(z)
        if s == 63 and 8 - k < 8:
            z = nc.sync.dma_start(t[(8 - k) * 12:96, :], zt[0:k * 12, :])
            for a in (a1, z0):
                add_dep(z, a, reason="zero-pad-rows")
            insts.append(z)
        store[s] = (t, insts)

    def halo(store, s, eng):
        # store[s][96:120] <- store[s+1][0:24]  (rows y+8, y+9)
        dst, insts = store[s]
        if s + 1 in store:
            d = eng.dma_start(dst[96:120, :], store[s + 1][0][0:24, :])
        else:
            d = eng.dma_start(dst[96:120, :], zt[0:24, :])
        for i_ in insts:
            add_dep(d, i_, reason="halo-after-evict")

    ph2_t = {}

    def phase2a(s, pt):
        # DVE/scalar part: i = relu(f3), E = exp(relu(w3)); the selector
        # matmuls run one iteration later (phase2b) so they never make the
        # PE queue wait on this cross-engine chain
        it = ph2.tile([96, 512], BF16, tag="i")
        et = ph2.tile([96, 512], BF16, tag="e")
        nc.vector.tensor_scalar_max(it[:], pt[:, 0:512], 0.0)
        nc.scalar.activation(et[:], pt[:, 512:1024], EXP)
        # exp(relu(x)) == max(exp(x), 1) -- this IS the last-layer relu
        # (gpsimd tensor ops are ~15x slower than DVE -- keep on vector)
        nc.vector.tensor_scalar_max(et[:], et[:], 1.0)
        nc.vector.tensor_mul(it[:], it[:], et[:])  # i*E in place
        ph2_t[s] = (it, et)

    def phase2b(s):
        it, et = ph2_t.pop(s)
        p2 = ps2.tile([24, 512], F32, tag="p2")
        nc.tensor.matmul(p2[:], selA_v, it[:], start=True, stop=False)
        nc.tensor.matmul(p2[:], selB_v, et[:], start=False, stop=True)
        s2 = ph2.tile([24, 512], F32, tag="s2")
        nc.scalar.copy(s2[:], p2[:])
        ys = 8 * s + 3
        ya, yb = max(ys, 0), min(ys + 8, HW)
        while ya < yb:
            run = min(yb - ya, 128 - (ya % 128))
            p0 = ya % 128
            dst = g3[p0:p0 + run, :].rearrange(
                "p (s c x) -> p s c x", s=3, c=4)[:, :, ya // 128, :]
            sv = s2[(ya - ys) * 3:(ya - ys + run) * 3, :]
            nc.gpsimd.dma_start(dst, sv)
            ya += run

    # --- phases 2.5/3/4 as chunked functions, drip-fed into the wavefront
    asm_specs = [  # (ch, py, px, qp index or None, plane addend)
        (0, 0, 0, 0, 0), (0, 0, 1, None, 1),
        (0, 1, 0, 4, "g1"), (0, 1, 1, 3, 3),
        (1, 0, 0, None, 0), (1, 0, 1, None, "g0"),
        (1, 1, 0, None, "g1"), (1, 1, 1, None, 3),
        (2, 0, 0, 2, 0), (2, 0, 1, 5, "g0"),
        (2, 1, 0, None, 2), (2, 1, 1, 1, 3),
    ]
    qp6_h = {}

    def phase25(hc):
        # green + chroma for y rows 64*hc .. 64*hc+63
        t = hc // 2
        p0 = (hc % 2) * 64
        cs = slice(t * 512, (t + 1) * 512)
        pr = slice(p0, p0 + 64)
        rd = rden[pr, 0:512]
        nc.vector.reciprocal(rd, den[pr, cs])
        nc.vector.tensor_mul(g0p[pr, cs], g0n[pr, cs], rd)
        nc.vector.tensor_mul(g1p[pr, cs], g1n[pr, cs], rd)
        nc.vector.tensor_sub(c1p[pr, cs], mos_pl(1)[pr, cs], g0p[pr, cs])
        nc.vector.tensor_sub(c2p[pr, cs], mos_pl(2)[pr, cs], g1p[pr, cs])

    p3_state = {}

    def phase3_prep(sq):
        # gather the 18-row (c1,c2) band into b3 well before its matmuls
        b3 = pools["b3"].tile([64, 514], BF16, tag="b3", name="b3")
        wrs = [nc.gpsimd.memset(b3[0:64, 0:514:513], 0.0)]
        y0 = 16 * sq - 1
        if y0 < 0:
            wrs.append(nc.gpsimd.memset(b3[0:1, :], 0.0))
            wrs.append(nc.gpsimd.memset(b3[32:33, :], 0.0))
        if y0 + 18 > HW:
            wrs.append(nc.sync.dma_start(b3[17:18, :], zt[0:1, 0:514]))
            wrs.append(nc.sync.dma_start(b3[49:50, :], zt[0:1, 0:514]))
        for cc, src_plane in ((0, c1p), (1, c2p)):
            ya, yb = max(y0, 0), min(y0 + 18, HW)
            while ya < yb:
                run = min(yb - ya, 128 - (ya % 128))
                d = nc.sync.dma_start(
                    b3[cc * 32 + ya - y0:cc * 32 + ya - y0 + run, 1:513],
                    src_plane[ya % 128:ya % 128 + run,
                              (ya // 128) * 512:(ya // 128) * 512 + 512])
                wrs.append(d)
                ya += run
        wrs.append(nc.gpsimd.dma_start(b3[18:32, :], zt[0:14, 0:514]))
        wrs.append(nc.gpsimd.dma_start(b3[50:64, :], zt[0:14, 0:514]))
        p3_state[sq] = (b3, wrs)

    def phase3_strip(sq):
        h = sq // 16
        if h not in qp6_h:
            qp6_h[h] = qpp.tile([128, 6 * 1024], BF16, tag="qp6",
                                name=f"qp6_{h}")
        qp6 = qp6_h[h]
        b3, wrs = p3_state.pop(sq)
        p3 = ps3.tile([96, 512], F32, tag="p3", name="p3")
        mm3 = [nc.tensor.matmul(p3[:], w5_v(dx), b3[0:64, dx:dx + 512],
                                start=(dx == 0), stop=(dx == 2))
               for dx in range(3)]
        for mm in mm3:
            for wr in wrs:
                add_dep(mm, wr, reason="b3-ready")
        s3 = ph2.tile([96, 512], BF16, tag="s3")
        nc.scalar.copy(s3[:], p3[:])
        yq = 16 * sq
        tlc = (yq // 128) - 2 * h  # 0 or 1: 512-chunk within the half
        dst = qp6[yq % 128:yq % 128 + 16, :].rearrange(
            "p (o c x) -> p o c x", o=6, c=2)[:, :, tlc, :]
        nc.sync.dma_start(dst, s3[:])

    def asm_unit(t, ch, py):
        # one output plane [128 quad rows, 1024 cols] for plane-chunk t
        h, tl = t // 2, t % 2
        qp6 = qp6_h[h]
        a = asmp.tile([128, 1024], F32, tag="asm", name="asm")
        prev = None
        for (c_, py_, px, qo, addend) in asm_specs:
            if c_ != ch or py_ != py:
                continue
            if addend == "g0":
                ad = g0p[:, t * 512:(t + 1) * 512]
            elif addend == "g1":
                ad = g1p[:, t * 512:(t + 1) * 512]
            else:
                ad = mos_pl(addend)[:, t * 512:(t + 1) * 512]
            view = a[:].rearrange("p (x two) -> p two x", two=2)[:, px, :]
            if qo is None:
                w_ = nc.vector.tensor_copy(view, ad)
            else:
                w_ = nc.vector.tensor_add(
                    view,
                    qp6[:, qo * 1024 + tl * 512:qo * 1024 + tl * 512 + 512],
                    ad)
            if prev is not None:
                add_dep(w_, prev, reason="asm-interleave")
            prev = w_
        dst = out[ch].rearrange("(y two) x -> two y x", two=2)[
            py, t * 128:(t + 1) * 128, :]
        nc.sync.dma_start(dst, a[:])

    # ---------------- emission schedule
    # a FIFO of small background closures (phase25 pieces, phase3 strips,
    # assembly units) drip-fed into the wavefront: bounded per-iteration
    # engine load, so the PE never starves (HAM stays at full clock)
    import collections
    bg = collections.deque()  # entries: (ready_iteration, closure)
    done3 = -1
    emitted3 = set()
    cur_i = [0]

    def phase25_parts(row0, nrows):
        # green + chroma for y rows row0 .. row0+nrows-1 (within one chunk).
        # reciprocal is split in 4 so no single DVE op blocks the queue
        t = row0 // 128
        p0 = row0 % 128
        cs = slice(t * 512, (t + 1) * 512)
        pr = slice(p0, p0 + nrows)
        rd = rden[pr, 0:512]

        def recip(j):
            def f():
                nc.vector.reciprocal(
                    rd[:, j * 128:(j + 1) * 128],
                    den[pr, t * 512 + j * 128:t * 512 + (j + 1) * 128])
            return f

        def part_b():
            nc.vector.tensor_mul(g0p[pr, cs], g0n[pr, cs], rd)
            nc.vector.tensor_sub(c1p[pr, cs], mos_pl(1)[pr, cs], g0p[pr, cs])

        def part_c():
            nc.vector.tensor_mul(g1p[pr, cs], g1n[pr, cs], rd)
            nc.vector.tensor_sub(c2p[pr, cs], mos_pl(2)[pr, cs], g1p[pr, cs])

        return [recip(0), recip(1), recip(2), recip(3), part_b, part_c]

    # phase25/unlock trigger table: after phase2b(t3b), rows <= 8*t3b+10
    # are in the green planes. Chunks 0-2 go in 64-row pieces; chunk 3 in
    # 16-row pieces so the last phase3 strips start before the wavefront ends.
    p25_trig = {}
    for hc_ in range(7):
        p25_trig[8 * hc_ + 7] = (64 * hc_, 64, 4 * hc_ + 2)
    # (compute-engine APs must start at a multiple-of-32 partition)
    p25_trig[59] = (448, 32, 28)
    p25_trig[63] = (480, 32, 31)

    def note_phase3_done(sq):
        nonlocal done3
        emitted3.add(sq)
        while done3 + 1 in emitted3:
            done3 += 1
            if (done3 + 1) % 8 == 0:
                t = done3 // 8
                units = [(ch, py) for ch in range(3) for py in range(2)]
                for idx, (ch, py) in enumerate(units):
                    bg.append((cur_i[0] + 1 + idx // 2,
                               lambda t=t, ch=ch, py=py: asm_unit(t, ch, py)))

    def unlock(lim):
        # preps go two iterations out (past the phase25 DVE drain); each
        # strip's matmuls one iteration after its prep, max one strip of
        # matmuls per iteration -- so phase3 MMs never camp at the head of
        # the PE queue waiting on gathers or the previous strip's eviction
        nonlocal sq_next
        batch = list(range(sq_next, lim + 1))
        sq_next = lim + 1
        base = cur_i[0]
        for idx, sq in enumerate(batch):
            bg.append((base + 2 + idx // 2, lambda sq=sq: phase3_prep(sq)))
        for idx, sq in enumerate(batch):
            def do3(sq=sq):
                phase3_strip(sq)
                note_phase3_done(sq)
            bg.append((base + 3 + idx, do3))

    sq_next = 0
    load_b0(-1)

    for i in range(NSTRIP + 4):
        cur_i[0] = i
        s = i - 1  # L1 strip index
        if s + 1 <= 63:
            load_b0(s + 1)  # prefetch one iteration ahead
        if s <= 63:
            pt = conv_pair(b0_t[s][0], 40, lambda dx: w1_v(0, dx),
                           lambda dx: w1_v(1, dx), 0, deps=b0_t[s][1])
            evict_pair(pt, b1_t, s, "b1", 1)
            b0_t.pop(s - 1, None)
        # halo for b1[s-1] (consumed by L2 next iteration); for s == 64 the
        # source strip 64 doesn't exist -> zero fill via halo() fallback
        if s - 1 in b1_t:
            halo(b1_t, s - 1, nc.sync)
        t2 = s - 2  # L2 strip index
        if -1 <= t2 <= 63:
            bt = b1_t[t2][0]
            pt = conv_pair(bt, 120, lambda dx: w23_v(0, dx),
                           lambda dx: w23_v(1, dx), 514)
            evict_pair(pt, b2_t, t2, "b2", 2)
        if t2 - 1 in b2_t:
            halo(b2_t, t2 - 1, nc.gpsimd)
        t3 = s - 4  # L3 strip index
        if -1 <= t3 <= 63:
            bt = b2_t[t3][0]
            pt = conv_pair(bt, 120, lambda dx: w23_v(2, dx),
                           lambda dx: w23_v(3, dx), 514)
            phase2a(t3, pt)
            b1_t.pop(t3, None)
            b2_t.pop(t3 - 1, None)
        t3b = t3 - 1  # deferred selector matmuls + plane write
        if -1 <= t3b <= 63:
            phase2b(t3b)
            if t3b in p25_trig:
                row0, nrows, lim = p25_trig[t3b]
                bg.extend((i, f) for f in phase25_parts(row0, nrows))
                unlock(lim)
        # drip background closures (FIFO; head must be past its ready-iter)
        for _ in range(3):
            if bg and bg[0][0] <= i:
                bg.popleft()[1]()

    phase2b(63)
    row0, nrows, lim = p25_trig[63]
    bg.extend((0, f) for f in phase25_parts(row0, nrows))
    unlock(lim)
    # drain whatever is left (tail) -- no budget, engines pipeline freely
    while bg:
        bg.popleft()[1]()

    if "dbgp" in outs:
        dbgp = outs["dbgp"]  # [4, 512, 512] bf16
        for j, pl in enumerate((g0p, g1p, c1p, c2p)):
            nc.sync.dma_start(
                dbgp[j].rearrange("(t p) x -> p t x", p=128),
                pl.rearrange("p (t x) -> p t x", t=4))
        dbgq = outs["dbgq"]  # [2, 128, 6144] bf16
        for j, h in enumerate(sorted(qp6_h)):
            nc.sync.dma_start(dbgq[j], qp6_h[h][:])


_CACHE = {}


def _get_compiled():
    if "nc" in _CACHE:
        return _CACHE["nc"]
    nc = bacc.Bacc("TRN2", target_bir_lowering=False, debug=False,
                   enable_asserts=False)
    ins = {
        "mospad": nc.dram_tensor("mospad", [4, 522, 514], BF16,
                                 kind="ExternalInput").ap(),
        "wblob": nc.dram_tensor("wblob", [120, WBLOB_COLS], BF16,
                                kind="ExternalInput").ap(),
    }
    outs = {"out": nc.dram_tensor("out", [3, 1024, 1024], F32,
                                  kind="ExternalOutput").ap()}
    from contextlib import ExitStack
    with tile.TileContext(nc) as tc, ExitStack() as ctx:
        build_kernel(tc, outs, ins, ctx)
    nc.compile()
    _CACHE["nc"] = nc
    return nc


def kernel(**inputs):
    nc = _get_compiled()
    mospad, shared = _host_prep(inputs)
    in_maps = []
    for b in range(8):
        m = {"mospad": np.ascontiguousarray(mospad[b])}
        m.update(shared)
        in_maps.append(m)
    res = run_bass_kernel_spmd(nc, in_maps, core_ids=list(range(8)))
    return np.stack([res.results[b]["out"] for b in range(8)])
